# revision 1
# baseline (speedup 1.0000x reference)
"""CrossTransformer kernel v4 for Trainium2 — fp8 sim-direct, direct-V^T.

Per batch b (B=32 -> 4/core, N=25, C=512, H=W=14, DK=DV=128):
  qq = Wqk @ Q   (bf16)        qv = Wv @ Q  (bf16 -> f32)
  qk = Wqk^T @ qq  -> fp8 e4m3 DoubleRow layout [g][p][t][hw], c = g*256+t*128+p
  sim[nij,hw] = S^T @ qk       (fp8 DoubleRow; S is never projected to K)
  E = exp(sim) bf16            (ACT, 392-wide PSUM chunk pairs)
  V^T[nij,dv] = S^T @ (16*Wv)^T  (fp8 DoubleRow, direct transposed layout)
  ctx_raw[hw,132] = sum_j E_j^T @ [V^T_j | 16]   (ones=16 cancels the Wv scale)
  partial += sum((qv^T - num*recip(den))^2)

Schedule: iteration b emits sim(b) interleaved with V^T(b+1), qk(b+1) and
PV(b) (both halves, 2 matmuls per slot, lagging the exp stream by 1 slot;
tails carry into iteration b+1) so the PE never head-of-line blocks on exp
or PSUM drains; the ACT exp stream (~10.4us/batch) is the pacing engine.
GPSIMD touches only SBUF (PSUM access is rejected by the BIR verifier) and
DoubleRow operand k-tile strides are padded to 16B alignment (4912/208).
"""

import os
import sys

sys.path.insert(0, "/opt/trn_rl_repo")

import numpy as np
import ml_dtypes

import concourse.bass as bass
import concourse.bacc as bacc
import concourse.mybir as mybir
import concourse.tile as tile
from concourse.bass_utils import run_bass_kernel_spmd
from concourse.masks import make_identity

F32 = mybir.dt.float32
BF16 = mybir.dt.bfloat16
FP8 = mybir.dt.float8e4

B_PER_CORE = 4
N_SUP = 25
C = 512
HW = 196
NIJ = N_SUP * HW          # 4900
DK = 128
NCH = 39                  # nij chunks of <=128 (last = 36 rows)
NIJP = 4912               # s8 SBUF row pitch: 16B-aligned for DoubleRow
HWP = 208                 # qk8 row pitch: 16B-aligned for DoubleRow
NPAIR = 20                # sim chunk pairs (last = chunk 38 alone)
VSCALE = 16.0             # host scales Wv by 16; ones column = 16 cancels it

DR = mybir.MatmulPerfMode.DoubleRow
EXP = mybir.ActivationFunctionType.Exp
RCP = mybir.ActivationFunctionType.Reciprocal
MULT = mybir.AluOpType.mult
SUBTRACT = mybir.AluOpType.subtract
ADD = mybir.AluOpType.add


def build_bass():
    nc = bacc.Bacc(
        "TRN2", target_bir_lowering=False, debug=False, enable_asserts=False
    )
    s8_d = nc.dram_tensor(
        "s8", [B_PER_CORE, 2, 128, 2, NIJ], FP8, kind="ExternalInput"
    ).ap()
    # packed: [p, cc0..3]=WqkT chunks, [p, 4..7]=WvT chunks
    wT_d = nc.dram_tensor("wT", [128, 8, DK], BF16, kind="ExternalInput").ap()
    q_d = nc.dram_tensor(
        "qbf", [128, 4, B_PER_CORE * HW], BF16, kind="ExternalInput"
    ).ap()
    wqk_d = nc.dram_tensor("wqk", [DK, C], BF16, kind="ExternalInput").ap()
    wv8_d = nc.dram_tensor("wv8", [128, 2, 2, DK], FP8, kind="ExternalInput").ap()
    out_d = nc.dram_tensor(
        "out", [128, 2 * B_PER_CORE], F32, kind="ExternalOutput"
    ).ap()

    with tile.TileContext(nc) as tc:
        with (
            tc.tile_pool(name="const", bufs=1) as const,
            tc.tile_pool(name="s8p", bufs=8) as s8p,
            tc.tile_pool(name="etp", bufs=2) as etp,
            tc.tile_pool(name="vtp", bufs=3) as vtp,
            tc.tile_pool(name="qk8p", bufs=4) as qk8p,
            tc.tile_pool(name="small", bufs=8) as small,
            tc.tile_pool(name="ps_sim", bufs=4, space="PSUM") as ps_sim,
            tc.tile_pool(name="ps_vt", bufs=2, space="PSUM") as ps_vt,
            tc.tile_pool(name="ps_ctx", bufs=2, space="PSUM") as ps_ctx,
        ):
            # ---- input DMAs, ordered for fastest time-to-first-exp ----
            wT_sb = const.tile([128, 8, DK], BF16, tag="wT_sb")
            nc.sync.dma_start(out=wT_sb, in_=wT_d)
            q_sb = const.tile([128, 4, B_PER_CORE * HW], BF16, tag="q_sb")
            nc.sync.dma_start(out=q_sb[:, 0:2], in_=q_d[:, 0:2])
            nc.sync.dma_start(out=q_sb[:, 2:4], in_=q_d[:, 2:4])
            wqk_sb = const.tile([128, C], BF16, tag="wqk_sb")
            nc.sync.dma_start(out=wqk_sb, in_=wqk_d)

            s8 = {}

            def s8_alloc(b):
                for g in range(2):
                    s8t = s8p.tile([128, 2, NIJP], FP8, tag="s8")
                    s8[(b, g)] = s8t

            def s8_piece(b, o, ln):
                for g in range(2):
                    nc.sync.dma_start(
                        out=s8[(b, g)][:, :, o : o + ln],
                        in_=s8_d[b, g][:, :, o : o + ln],
                    )

            def s8_dma(b, pieces):
                s8_alloc(b)
                w = NIJ // pieces
                for i in range(pieces):
                    o = i * w
                    s8_piece(b, o, w if i < pieces - 1 else NIJ - o)

            # prologue pieces ordered so sim(0)/vt(0)/vt(1) prerequisites
            # land just in time (HWDGE serializes at 625ns per DMA)
            s8_alloc(0)
            s8_alloc(1)
            s8_piece(0, 0, 1225)
            wv8_sb = const.tile([128, 2, 2, DK], FP8, tag="wv8_sb")
            nc.sync.dma_start(out=wv8_sb, in_=wv8_d)
            s8_piece(1, 0, 2450)
            s8_piece(0, 1225, 1225)
            s8_piece(0, 2450, 2450)
            s8_piece(1, 2450, 2450)

            # PE p-state warmup: wide matmuls on a zero tile keep the PE
            # continuously busy through the DMA wait so the real prologue
            # runs at full clock (ap 512 > write latency -> no WAW stall)
            warm_src = const.tile([128, 512], BF16, tag="warm_src")
            nc.gpsimd.memset(warm_src, 0.0)
            for i in range(12):
                pw = ps_vt.tile([128, 512], F32, tag="ps_vt")
                nc.tensor.matmul(
                    pw,
                    lhsT=warm_src[:, 0:128],
                    rhs=warm_src,
                    start=True,
                    stop=True,
                )


            # ---- qq projection (all 4 batches at once) ----
            qq_bf = const.tile([128, B_PER_CORE * HW], BF16, tag="qq_bf")

            def q_proj(wo, dst, eng):
                for half in range(2):
                    hw0 = half * 392
                    pq = ps_sim.tile([128, 392], F32, tag="ps_sim")
                    for cc in range(4):
                        nc.tensor.matmul(
                            pq,
                            lhsT=wT_sb[:, wo + cc],
                            rhs=q_sb[:, cc, hw0 : hw0 + 392],
                            start=(cc == 0),
                            stop=(cc == 3),
                        )
                    eng.tensor_copy(dst[:, hw0 : hw0 + 392], pq)

            qvT = {}

            def qvT_prep(b, h):
                """qv^T[hw, dk] computed directly: lhsT = Q chunk, rhs = Wv^T."""
                hww = 128 if h == 0 else HW - 128
                o = b * HW + h * 128
                pt = ps_sim.tile([128, 392], F32, tag="ps_sim")
                for cc in range(4):
                    nc.tensor.matmul(
                        pt[:hww, 0:128],
                        lhsT=q_sb[:, cc, o : o + hww],
                        rhs=wT_sb[:, 4 + cc],
                        start=(cc == 0),
                        stop=(cc == 3),
                    )
                qt = const.tile([128, 128], F32, tag=f"qvT{b}_{h}")
                nc.vector.tensor_copy(qt[:hww, :], pt[:hww, 0:128])
                qvT[(b, h)] = qt


            # ---- per-batch stage generators (interleavable) ----
            et = {}
            vt1 = {}
            qk8 = {}

            def qk_prep(b):
                """qk = Wqk^T @ qq -> fp8 DoubleRow layout [128, 2(g), 2(t), 196]."""
                k8 = qk8p.tile([128, 2, 2, HWP], FP8, tag="qk8")
                for g in range(2):
                    pk = ps_sim.tile([128, 392], F32, tag="ps_sim")
                    for t in range(2):
                        cc = 2 * g + t
                        nc.tensor.matmul(
                            pk[:, t * HW : (t + 1) * HW],
                            lhsT=wqk_sb[:, cc * 128 : (cc + 1) * 128],
                            rhs=qq_bf[:, b * HW : (b + 1) * HW],
                            start=True,
                            stop=True,
                        )
                    nc.vector.tensor_copy(
                        k8[:, g, :, 0:HW],
                        pk.rearrange("p (t hw) -> p t hw", t=2),
                    )
                qk8[b] = k8

            def sim_pair_gen(b):
                """Yield once per sim chunk pair: 4 matmuls + 1 exp on ACT."""
                e = etp.tile([128, NCH * HW], BF16, tag="et")
                nc.gpsimd.memset(e[:, 38 * HW :], 0.0)
                et[b] = e
                for jp in range(NPAIR):
                    chunks = (2 * jp, 2 * jp + 1) if jp < NPAIR - 1 else (38,)
                    ps = ps_sim.tile([128, 392], F32, tag="ps_sim")
                    for ci, j in enumerate(chunks):
                        cw = min(128, NIJ - j * 128)
                        for g in range(2):
                            nc.tensor.matmul(
                                ps[:cw, ci * HW : (ci + 1) * HW],
                                lhsT=s8[(b, g)][:, :, j * 128 : j * 128 + cw],
                                rhs=qk8[b][:, g, :, 0:HW],
                                start=(g == 0),
                                stop=(g == 1),
                                perf_mode=DR,
                            )
                    if jp < NPAIR - 1:
                        nc.scalar.activation(
                            out=e[:, 2 * jp * HW : (2 * jp + 2) * HW],
                            in_=ps,
                            func=EXP,
                        )
                    else:
                        nc.scalar.activation(
                            out=e[:36, 38 * HW :], in_=ps[:36, 0:HW], func=EXP
                        )
                    yield

            def vt_alloc(b):
                vt = vtp.tile([128, NCH * 132], BF16, tag="vt1")
                nc.gpsimd.memset(
                    vt.rearrange("p (j c) -> p j c", j=NCH)[:, :, 128:132], VSCALE
                )
                vt1[b] = vt

            def vt_gen(b, j_lo, j_hi, copy_eng0):
                """Yield per V^T chunk: 2 DoubleRow matmuls into a [128,512]
                PSUM quad tile; one wide copy per quad."""
                vtr = vt1[b].rearrange("p (j c) -> p j c", j=NCH)
                ncopy = copy_eng0
                for j0 in range(j_lo, j_hi, 4):
                    jn = min(4, j_hi - j0)
                    pq = ps_vt.tile([128, 512], F32, tag="ps_vt")
                    for ji in range(jn):
                        j = j0 + ji
                        cw = min(128, NIJ - j * 128)
                        for g in range(2):
                            nc.tensor.matmul(
                                pq[:cw, ji * 128 : (ji + 1) * 128],
                                lhsT=s8[(b, g)][:, :, j * 128 : j * 128 + cw],
                                rhs=wv8_sb[:, g],
                                start=(g == 0),
                                stop=(g == 1),
                                perf_mode=DR,
                            )
                        yield
                    nc.vector.tensor_copy(
                        vtr[:, j0 : j0 + jn, 0:128],
                        pq[:, 0 : jn * 128].rearrange("p (j c) -> p j c", j=jn),
                    )
                    ncopy += 1

            def pv_half_gen(b, h):
                """Yield per PV matmul (one et chunk each)."""
                vtr = vt1[b].rearrange("p (j c) -> p j c", j=NCH)
                hww = 128 if h == 0 else HW - 128
                pc = ps_ctx.tile([128, 132], F32, tag="ps_ctx")
                for j in range(NCH):
                    nc.tensor.matmul(
                        pc[:hww, :],
                        lhsT=et[b][:, j * HW + h * 128 : j * HW + h * 128 + hww],
                        rhs=vtr[:, j],
                        start=(j == 0),
                        stop=(j == NCH - 1),
                    )
                    yield
                # ctx epilogue avoids DVE entirely: late-run DVE queues behind
                # the tile framework's semaphore range-clear barriers.
                r = small.tile([128, 1], F32, tag="recip")
                nc.vector.reciprocal(r[:hww], pc[:hww, 128:129])
                d = small.tile([128, 128], F32, tag="diff")
                nc.vector.scalar_tensor_tensor(
                    d[:hww, :],
                    pc[:hww, 0:128],
                    r[:hww],
                    qvT[(b, h)][:hww, :],
                    op0=MULT,
                    op1=SUBTRACT,
                )
                d2 = small.tile([128, 128], F32, tag="d2")
                nc.vector.scalar_tensor_tensor(
                    d2[:hww, :],
                    d[:hww, :],
                    1.0,
                    d[:hww, :],
                    op0=MULT,
                    op1=MULT,
                    accum_out=partials[:hww, 2 * b + h : 2 * b + h + 1],
                )

            def drain(gen, n=None):
                if gen is None:
                    return None
                try:
                    if n is None:
                        while True:
                            next(gen)
                    else:
                        for _ in range(n):
                            next(gen)
                except StopIteration:
                    return None
                return gen

            # ---- schedule ----
            # PV(b) runs in-iteration, lagging its own exp stream by 2 slots;
            # its last 3 matmuls + ctx epilogue carry into iteration b+1.
            partials = const.tile([128, 2 * B_PER_CORE], F32, tag="partials")
            nc.vector.memset(partials, 0.0)
            q_proj(0, qq_bf, nc.vector)
            qk_prep(0)
            for i, (bq, hq) in enumerate(
                (b, h) for b in range(B_PER_CORE) for h in range(2)
            ):
                qvT_prep(bq, hq)
            vt_alloc(0)
            drain(vt_gen(0, 0, 19, 0))   # chunks covered by first two s8 quarters
            vt0_rest = vt_gen(0, 19, NCH, 1)
            carry = []

            for b in range(B_PER_CORE):
                simg = sim_pair_gen(b)
                if b + 1 < B_PER_CORE:
                    vt_alloc(b + 1)
                    vtg = vt_gen(b + 1, 0, NCH, 0)
                else:
                    vtg = None
                pvg = [pv_half_gen(b, h) for h in range(2)]
                for jp in range(NPAIR):
                    simg = drain(simg, 1)
                    if b == 0:
                        vt0_rest = drain(vt0_rest, 2)
                    if jp < 2 and carry:
                        carry = [drain(g, 2) for g in carry if g is not None]
                    if (b > 0 and jp >= 1) or jp >= 2:
                        vtg = drain(vtg, 3 if b == 0 else 2)
                    if jp >= 2:
                        pvg = [drain(g, 2) for g in pvg]
                    if jp == 0 and b + 2 < B_PER_CORE:
                        s8_dma(b + 2, 4)
                    if jp == 9 and b + 1 < B_PER_CORE:
                        qk_prep(b + 1)
                drain(simg)
                drain(vtg)
                if b == 0:
                    drain(vt0_rest)
                for g in carry:
                    drain(g)
                carry = [g for g in pvg if g is not None]
            for g in carry:
                drain(g)

            # final reduction happens on the host: DMA the partials matrix
            nc.sync.dma_start(out=out_d, in_=partials)

    nc.compile()
    return nc


_NC = None


def _prep_core(q, s, b0):
    """Host-side per-core input layouts (cast + transpose only)."""
    F8NP = ml_dtypes.float8_e4m3
    BFNP = ml_dtypes.bfloat16
    sb = s[b0 : b0 + B_PER_CORE]                      # [4, 25, 512, 196]
    s_c = sb.transpose(0, 2, 1, 3).reshape(B_PER_CORE, 2, 2, 128, NIJ)
    s8 = np.ascontiguousarray(s_c.transpose(0, 1, 3, 2, 4)).astype(F8NP)
    qb = q[b0 : b0 + B_PER_CORE]                      # [4, 512, 196]
    qbf = np.ascontiguousarray(
        qb.reshape(B_PER_CORE, 4, 128, HW).transpose(2, 1, 0, 3).reshape(
            128, 4, B_PER_CORE * HW
        )
    ).astype(BFNP)
    return {"s8": s8, "qbf": qbf}


def kernel(query_repr, supports_repr, W_qk, W_v):
    global _NC
    F8NP = ml_dtypes.float8_e4m3
    BFNP = ml_dtypes.bfloat16

    q = np.asarray(query_repr, dtype=np.float32).reshape(32, C, HW)
    s = np.asarray(supports_repr, dtype=np.float32).reshape(32, N_SUP, C, HW)
    wqk = np.asarray(W_qk, dtype=np.float32)
    wv = np.asarray(W_v, dtype=np.float32)

    wqk_bf = wqk.astype(BFNP)
    # wT[p, 0:4, dk] = Wqk^T chunks, wT[p, 4:8, dk] = Wv^T chunks
    wT = np.concatenate(
        [wqk.T.reshape(4, 128, DK), wv.T.reshape(4, 128, DK)], axis=0
    ).transpose(1, 0, 2)
    wT = np.ascontiguousarray(wT).astype(BFNP)
    wv8 = np.ascontiguousarray(
        (VSCALE * wv).T.reshape(2, 2, 128, DK).transpose(2, 0, 1, 3)
    ).astype(F8NP)

    if _NC is None:
        _NC = build_bass()

    in_maps = []
    for core in range(8):
        m = _prep_core(q, s, core * B_PER_CORE)
        m.update({"wqk": wqk_bf, "wT": wT, "wv8": wv8})
        in_maps.append(m)
    res = run_bass_kernel_spmd(
        _NC, in_maps, core_ids=list(range(8)),
        trace=bool(int(os.environ.get("KTRACE", "0"))),
    )
    total = sum(float(r["out"].astype(np.float64).sum()) for r in res.results)
    total = total / float(HW)
    kernel._last_results = res
    return np.asarray(total, dtype=np.float32)



# revision 41
# speedup vs baseline: 1.0312x; 1.0312x over previous
"""CrossTransformer kernel v5 for Trainium2 — fp8 sim-direct, quad-fused exp.

Per batch b (B=32 -> 4/core, N=25, C=512, H=W=14, DK=DV=128):
  qq = Wqk @ Q   (bf16)        qv = Wv @ Q  (bf16 -> f32)
  qk = Wqk^T @ qq  -> fp8 e4m3 DoubleRow layout [g][p][t][hw], c = g*256+t*128+p
  sim[nij,hw] = S^T @ qk       (fp8 DoubleRow; S is never projected to K)
  E = exp(sim) bf16            (ACT, quad-fused: one exp per 2 PSUM banks)
  V^T[nij,dv] = S^T @ (16*Wv)^T  (fp8 DoubleRow, direct transposed layout)
  ctx_raw[hw,129] = sum_j E_j^T @ [V^T_j | 16]   (ones=16 cancels the Wv scale)
  partial += sum((qv^T - num*recip(den))^2)

v5 vs v4: nij padded to 4992 on host (39 full 128-row chunks, no E memset);
sim PSUM tiles are 2-bank quads [128,2,512] so one ACT exp covers 784 cols
(init overhead amortized: 838ns/4chunks vs 1024); warmup/qk/qvT share the
ps_vt pool and ctx halves share a single bank tile to fit PSUM in 8 banks;
q_proj is split b0-first so qk8(0) unblocks the exp stream sooner.
GPSIMD touches only SBUF (PSUM access is rejected by the BIR verifier) and
DoubleRow operand k-tile strides are padded to 16B alignment (4992/208).
"""

import os
import sys

sys.path.insert(0, "/opt/trn_rl_repo")

import numpy as np
import ml_dtypes

import concourse.bass as bass
import concourse.bacc as bacc
import concourse.mybir as mybir
import concourse.tile as tile
from concourse.bass_utils import run_bass_kernel_spmd

F32 = mybir.dt.float32
BF16 = mybir.dt.bfloat16
FP8 = mybir.dt.float8e4

B_PER_CORE = 4
N_SUP = 25
C = 512
HW = 196
NIJ = N_SUP * HW          # 4900
DK = 128
NCH = 39                  # nij chunks of 128 (padded)
NIJP = NCH * 128          # 4992 padded nij: 16B-aligned for DoubleRow
HWP = 208                 # qk8 row pitch: 16B-aligned for DoubleRow
NQ = 10                   # sim quads per batch (last = 3 chunks)
VSCALE = 16.0             # host scales Wv by 16; ones column = 16 cancels it

DR = mybir.MatmulPerfMode.DoubleRow
EXP = mybir.ActivationFunctionType.Exp
MULT = mybir.AluOpType.mult
SUBTRACT = mybir.AluOpType.subtract


def build_bass():
    nc = bacc.Bacc(
        "TRN2", target_bir_lowering=False, debug=False, enable_asserts=False
    )
    s8_d = nc.dram_tensor(
        "s8", [B_PER_CORE, 2, 128, 2, NIJP], FP8, kind="ExternalInput"
    ).ap()
    wvT_d = nc.dram_tensor("wvT", [128, 4, DK], BF16, kind="ExternalInput").ap()
    q_d = nc.dram_tensor(
        "qbf", [128, B_PER_CORE, 4, HW], BF16, kind="ExternalInput"
    ).ap()
    # g8 = fp8(64 * Wqk^T Wqk) in DoubleRow lhsT layout [p, g, cc', t, m]
    g8_d = nc.dram_tensor(
        "g8", [128, 2, 4, 2, 128], FP8, kind="ExternalInput"
    ).ap()
    # q8 = fp8(Q) in DoubleRow rhs layout [p, b, g, t, hw]
    q8_d = nc.dram_tensor(
        "q8", [128, B_PER_CORE, 2, 2, HW], FP8, kind="ExternalInput"
    ).ap()
    wv8_d = nc.dram_tensor("wv8", [128, 2, 2, DK], FP8, kind="ExternalInput").ap()
    out_d = nc.dram_tensor(
        "out", [128, 2 * B_PER_CORE], F32, kind="ExternalOutput"
    ).ap()

    with tile.TileContext(nc) as tc:
        with (
            tc.tile_pool(name="const", bufs=1) as const,
            tc.tile_pool(name="s8p", bufs=8) as s8p,
            tc.tile_pool(name="etp", bufs=2) as etp,
            tc.tile_pool(name="vtp", bufs=3) as vtp,
            tc.tile_pool(name="qk8p", bufs=4) as qk8p,
            tc.tile_pool(name="small", bufs=8) as small,
            tc.tile_pool(name="ps_q", bufs=2, space="PSUM") as ps_q,
            tc.tile_pool(name="ps_vt", bufs=2, space="PSUM") as ps_vt,
            tc.tile_pool(name="ps_ctx", bufs=2, space="PSUM") as ps_ctx,
        ):
            # ---- input DMAs, ordered for fastest time-to-first-exp ----
            # Transfers FIFO-serialize at ~720B/ns in HWDGE issue order, so
            # the small q8/G8 pieces gating the qk chain go first on qSP
            # while wv8 interleaves from the qACT HWDGE queue.
            q8_sb = const.tile([128, B_PER_CORE, 2, 2, HW], FP8, tag="q8_sb")
            nc.sync.dma_start(out=q8_sb[:, 0], in_=q8_d[:, 0])
            g8_sb = const.tile([128, 2, 4, 2, 128], FP8, tag="g8_sb")
            nc.sync.dma_start(out=g8_sb, in_=g8_d)

            s8 = {}

            def s8_alloc(b):
                for g in range(2):
                    s8t = s8p.tile([128, 2, NIJP], FP8, tag="s8")
                    s8[(b, g)] = s8t

            def s8_piece(b, o, ln, eng=None):
                for g in range(2):
                    (eng or nc.sync).dma_start(
                        out=s8[(b, g)][:, :, o : o + ln],
                        in_=s8_d[b, g][:, :, o : o + ln],
                    )

            def s8_dma(b, pieces):
                s8_alloc(b)
                w = NIJP // pieces
                for i in range(pieces):
                    o = i * w
                    s8_piece(b, o, w if i < pieces - 1 else NIJP - o)

            s8_alloc(0)
            s8_alloc(1)
            wv8_sb = const.tile([128, 2, 2, DK], FP8, tag="wv8_sb")
            nc.scalar.dma_start(out=wv8_sb, in_=wv8_d)
            s8_piece(0, 0, 1280)
            s8_piece(0, 1280, 1280)
            wvT_sb = const.tile([128, 4, DK], BF16, tag="wvT_sb")
            nc.sync.dma_start(out=wvT_sb, in_=wvT_d)
            nc.sync.dma_start(out=q8_sb[:, 1:], in_=q8_d[:, 1:])
            q_sb = const.tile([128, B_PER_CORE, 4, HW], BF16, tag="q_sb")
            nc.sync.dma_start(out=q_sb[:, 0], in_=q_d[:, 0])
            s8_piece(0, 2560, 1280)
            s8_piece(0, 3840, 1152)
            nc.sync.dma_start(out=q_sb[:, 1:], in_=q_d[:, 1:])
            s8_piece(1, 0, 1248)
            s8_piece(1, 1248, 1248)
            s8_piece(1, 2496, 1248)
            s8_piece(1, 3744, 1248)

            # PE p-state warmup: wide matmuls on a zero tile start the PE
            # p-state ramp clock during the initial DMA wait so the real
            # prologue runs at or near full clock.
            warm_src = const.tile([128, 512], BF16, tag="warm_src")
            nc.vector.memset(warm_src, 0.0)
            for i in range(4):
                pw = ps_vt.tile([128, 512], F32, tag="ps_vt")
                nc.tensor.matmul(
                    pw,
                    lhsT=warm_src[:, 0:128],
                    rhs=warm_src,
                    start=True,
                    stop=True,
                )

            qvT = {}

            def qvT_prep(b, h):
                """qv^T[hw, dk] computed directly: lhsT = Q chunk, rhs = Wv^T."""
                hww = 128 if h == 0 else HW - 128
                o = h * 128
                pt = ps_vt.tile([128, 512], F32, tag="ps_vt")
                for cc in range(4):
                    nc.tensor.matmul(
                        pt[:hww, 0:128],
                        lhsT=q_sb[:, b, cc, o : o + hww],
                        rhs=wvT_sb[:, cc],
                        start=(cc == 0),
                        stop=(cc == 3),
                    )
                qt = const.tile([128, 128], F32, tag=f"qvT{b}_{h}")
                nc.vector.tensor_copy(qt[:hww, :], pt[:hww, 0:128])
                qvT[(b, h)] = qt

            # ---- per-batch stage generators (interleavable) ----
            et = {}
            vt1 = {}
            qk8 = {}

            def qk_prep(b):
                """qk = 64*(Wqk^T Wqk) @ Q via host G8, straight to fp8
                DoubleRow layout [128, 2(g), 2(t), 196]; the 64x scale is
                cancelled by the exp's 1/64 scale operand. One ps_q rotation
                turn; drained by ACT (a DVE drain here would stall the sim
                stream behind the DVE queue backlog)."""
                k8 = qk8p.tile([128, 2, 2, HWP], FP8, tag="qk8")
                pk = ps_q.tile([128, 2, 512], F32, tag="ps_q", name="pk")
                for go in range(2):
                    for t in range(2):
                        cc = 2 * go + t
                        for gi in range(2):
                            nc.tensor.matmul(
                                pk[:, go, t * HW : (t + 1) * HW],
                                lhsT=g8_sb[:, gi, cc],
                                rhs=q8_sb[:, b, gi],
                                start=(gi == 0),
                                stop=(gi == 1),
                                perf_mode=DR,
                            )
                nc.scalar.copy(
                    k8[:, :, :, 0:HW],
                    pk[:, :, 0 : 2 * HW].rearrange("p g (t hw) -> p g t hw", t=2),
                )
                qk8[b] = k8

            def sim_quad_gen(b):
                """Yield once per sim quad: 8 DR matmuls into a 2-bank PSUM
                tile + 1 (or 2, tail) quad-fused exp on ACT."""
                e = etp.tile([128, NCH * HW], BF16, tag="et")
                et[b] = e
                for jq in range(NQ):
                    chunks = range(4 * jq, min(4 * jq + 4, NCH))
                    ps = ps_q.tile([128, 2, 512], F32, tag="ps_q")
                    for ci, j in enumerate(chunks):
                        for g in range(2):
                            nc.tensor.matmul(
                                ps[
                                    :,
                                    ci // 2,
                                    (ci % 2) * HW : (ci % 2) * HW + HW,
                                ],
                                lhsT=s8[(b, g)][:, :, j * 128 : (j + 1) * 128],
                                rhs=qk8[b][:, g, :, 0:HW],
                                start=(g == 0),
                                stop=(g == 1),
                                perf_mode=DR,
                            )
                    o = 4 * jq * HW
                    if jq < NQ - 1:
                        nc.scalar.activation(
                            out=e[:, o : o + 4 * HW],
                            in_=ps[:, :, 0 : 2 * HW],
                            func=EXP,
                            scale=1.0 / 64.0,
                        )
                    else:
                        nc.scalar.activation(
                            out=e[:, o : o + 2 * HW],
                            in_=ps[:, 0, 0 : 2 * HW],
                            func=EXP,
                            scale=1.0 / 64.0,
                        )
                        nc.scalar.activation(
                            out=e[:, o + 2 * HW :],
                            in_=ps[:, 1, 0:HW],
                            func=EXP,
                            scale=1.0 / 64.0,
                        )
                    yield

            def vt_alloc(b):
                vt = vtp.tile([128, NCH * 132], BF16, tag="vt1")
                vtr = vt.rearrange("p (j c) -> p j c", j=NCH)
                # padded nij rows (chunk 38 rows 36:) must not count in den;
                # non-zero partition starts must be 0-based for the verifier
                nc.gpsimd.memset(vtr[:, 0 : NCH - 1, 128:132], VSCALE)
                nc.gpsimd.memset(vtr[:, NCH - 1, 128:132], 0.0)
                nc.gpsimd.memset(vtr[:36, NCH - 1, 128:132], VSCALE)
                vt1[b] = vt

            def vt_gen(b, j_lo, j_hi):
                """Yield per V^T chunk: 2 DoubleRow matmuls into a [128,512]
                PSUM quad tile; one wide copy per quad."""
                vtr = vt1[b].rearrange("p (j c) -> p j c", j=NCH)
                for j0 in range(j_lo, j_hi, 4):
                    jn = min(4, j_hi - j0)
                    pq = ps_vt.tile([128, 512], F32, tag="ps_vt")
                    for ji in range(jn):
                        j = j0 + ji
                        for g in range(2):
                            nc.tensor.matmul(
                                pq[:, ji * 128 : (ji + 1) * 128],
                                lhsT=s8[(b, g)][:, :, j * 128 : (j + 1) * 128],
                                rhs=wv8_sb[:, g],
                                start=(g == 0),
                                stop=(g == 1),
                                perf_mode=DR,
                            )
                        yield
                    nc.vector.tensor_copy(
                        vtr[:, j0 : j0 + jn, 0:128],
                        pq[:, 0 : jn * 128].rearrange("p (j c) -> p j c", j=jn),
                    )

            def ctx_alloc(b):
                return ps_ctx.tile([128, 2, 132], F32, tag="ps_ctx", name="pcb")

            def pv_half_gen(b, h, pcb):
                """Yield per PV matmul (one et chunk each)."""
                vtr = vt1[b].rearrange("p (j c) -> p j c", j=NCH)
                hww = 128 if h == 0 else HW - 128
                pc = pcb[:, h, 0:129]
                for j in range(NCH):
                    nc.tensor.matmul(
                        pc[:hww, :],
                        lhsT=et[b][:, j * HW + h * 128 : j * HW + h * 128 + hww],
                        rhs=vtr[:, j, 0:129],
                        start=(j == 0),
                        stop=(j == NCH - 1),
                    )
                    yield
                # ctx epilogue avoids DVE entirely: late-run DVE queues behind
                # the tile framework's semaphore range-clear barriers.
                r = small.tile([128, 1], F32, tag="recip")
                nc.vector.reciprocal(r[:hww], pc[:hww, 128:129])
                d = small.tile([128, 128], F32, tag="diff")
                nc.vector.scalar_tensor_tensor(
                    d[:hww, :],
                    pc[:hww, 0:128],
                    r[:hww],
                    qvT[(b, h)][:hww, :],
                    op0=MULT,
                    op1=SUBTRACT,
                )
                d2 = small.tile([128, 128], F32, tag="d2")
                nc.vector.scalar_tensor_tensor(
                    d2[:hww, :],
                    d[:hww, :],
                    1.0,
                    d[:hww, :],
                    op0=MULT,
                    op1=MULT,
                    accum_out=partials[:hww, 2 * b + h : 2 * b + h + 1],
                )

            def drain(gen, n=None):
                if gen is None:
                    return None
                try:
                    if n is None:
                        while True:
                            next(gen)
                    else:
                        for _ in range(n):
                            next(gen)
                except StopIteration:
                    return None
                return gen

            # ---- schedule ----
            # PV(b) runs in-iteration, lagging its own exp stream by 2 slots;
            # its tail + ctx epilogue carry into iteration b+1.
            partials = const.tile([128, 2 * B_PER_CORE], F32, tag="partials")
            nc.vector.memset(partials, 0.0)
            qk_prep(0)
            vt_alloc(0)
            drain(vt_gen(0, 0, 8))      # keeps the PE busy until qk8 lands
            vt0_rest = vt_gen(0, 8, NCH)

            def sim_chain():
                for b in range(B_PER_CORE):
                    yield from sim_quad_gen(b)

            simg = sim_chain()
            carry = []          # pv tails from the previous batch
            vt_carry = None     # vt tail from the previous batch
            pvg = []

            for s in range(B_PER_CORE * NQ):
                b, jq = divmod(s, NQ)
                simg = drain(simg, 1)
                if jq == 0:
                    pcb = ctx_alloc(b)
                    pvg = [pv_half_gen(b, h, pcb) for h in range(2)]
                    if b + 1 < B_PER_CORE:
                        vt_alloc(b + 1)
                        vtg = vt_gen(b + 1, 0, NCH)
                    else:
                        vtg = None
                    if b + 2 < B_PER_CORE:
                        s8_dma(b + 2, 4)
                if b == 0:
                    vt0_rest = drain(vt0_rest, 4)
                    if jq == 8:
                        qvT_prep(0, 0)
                        qvT_prep(0, 1)
                if jq < 2:
                    if vt_carry is not None:
                        vt_carry = drain(vt_carry, 3)
                    if carry:
                        carry = [drain(g, 4) for g in carry if g is not None]
                if b == 0:
                    if jq >= 5:
                        vtg = drain(vtg, 7)
                elif jq >= 1:
                    vtg = drain(vtg, 4)
                if jq >= 3:
                    pvg = [drain(g, 5) for g in pvg]
                if jq == 4 and b + 1 < B_PER_CORE:
                    qk_prep(b + 1)
                if jq == 6 and b + 1 < B_PER_CORE:
                    qvT_prep(b + 1, 0)
                    qvT_prep(b + 1, 1)
                if jq == NQ - 1:
                    vt_carry = vtg
                    carry = [g for g in pvg if g is not None]

            for g in carry:
                drain(g)

            # final reduction happens on the host: DMA the partials matrix
            nc.sync.dma_start(out=out_d, in_=partials)

    nc.compile()
    return nc


_NC = None


def _prep_core(q, s, b0):
    """Host-side per-core input layouts (cast + transpose + zero-pad only)."""
    F8NP = ml_dtypes.float8_e4m3
    BFNP = ml_dtypes.bfloat16
    sb = s[b0 : b0 + B_PER_CORE]                      # [4, 25, 512, 196]
    s_c = sb.transpose(0, 2, 1, 3).reshape(B_PER_CORE, 2, 2, 128, NIJ)
    s8 = np.zeros((B_PER_CORE, 2, 128, 2, NIJP), dtype=F8NP)
    s8[:, :, :, :, :NIJ] = s_c.transpose(0, 1, 3, 2, 4).astype(F8NP)
    qb = q[b0 : b0 + B_PER_CORE]                      # [4, 512, 196]
    qbf = np.ascontiguousarray(
        qb.reshape(B_PER_CORE, 4, 128, HW).transpose(2, 0, 1, 3)
    ).astype(BFNP)                                    # [128, b, cc, hw]
    q8 = np.ascontiguousarray(
        qb.reshape(B_PER_CORE, 2, 2, 128, HW).transpose(3, 0, 1, 2, 4)
    ).astype(F8NP)                                    # [128, b, g, t, hw]
    return {"s8": s8, "qbf": qbf, "q8": q8}


def kernel(query_repr, supports_repr, W_qk, W_v):
    global _NC
    F8NP = ml_dtypes.float8_e4m3
    BFNP = ml_dtypes.bfloat16

    q = np.asarray(query_repr, dtype=np.float32).reshape(32, C, HW)
    s = np.asarray(supports_repr, dtype=np.float32).reshape(32, N_SUP, C, HW)
    wqk = np.asarray(W_qk, dtype=np.float32)
    wv = np.asarray(W_v, dtype=np.float32)

    wvT = np.ascontiguousarray(
        wv.T.reshape(4, 128, DK).transpose(1, 0, 2)
    ).astype(BFNP)
    wv8 = np.ascontiguousarray(
        (VSCALE * wv).T.reshape(2, 2, 128, DK).transpose(2, 0, 1, 3)
    ).astype(F8NP)
    # G8 = 64 * Wqk^T Wqk (symmetric), fp8 DoubleRow lhsT layout
    g = 64.0 * (wqk.T @ wqk)                          # [512, 512]
    g8 = np.ascontiguousarray(
        g.reshape(2, 2, 128, 4, 128).transpose(2, 0, 3, 1, 4)
    ).astype(F8NP)                                    # [p, g, cc', t, m]

    if _NC is None:
        _NC = build_bass()

    in_maps = []
    for core in range(8):
        m = _prep_core(q, s, core * B_PER_CORE)
        m.update({"wvT": wvT, "wv8": wv8, "g8": g8})
        in_maps.append(m)
    res = run_bass_kernel_spmd(
        _NC, in_maps, core_ids=list(range(8)),
        trace=bool(int(os.environ.get("KTRACE", "0"))),
    )
    total = sum(float(r["out"].astype(np.float64).sum()) for r in res.results)
    total = total / float(HW)
    kernel._last_results = res
    return np.asarray(total, dtype=np.float32)


# revision 48
# speedup vs baseline: 1.0667x; 1.0344x over previous
"""CrossTransformer kernel v5 for Trainium2 — fp8 sim-direct, quad-fused exp.

Per batch b (B=32 -> 4/core, N=25, C=512, H=W=14, DK=DV=128):
  qq = Wqk @ Q   (bf16)        qv = Wv @ Q  (bf16 -> f32)
  qk = Wqk^T @ qq  -> fp8 e4m3 DoubleRow layout [g][p][t][hw], c = g*256+t*128+p
  sim[nij,hw] = S^T @ qk       (fp8 DoubleRow; S is never projected to K)
  E = exp(sim) bf16            (ACT, quad-fused: one exp per 2 PSUM banks)
  V^T[nij,dv] = S^T @ (16*Wv)^T  (fp8 DoubleRow, direct transposed layout)
  ctx_raw[hw,129] = sum_j E_j^T @ [V^T_j | 16]   (ones=16 cancels the Wv scale)
  partial += sum((qv^T - num*recip(den))^2)

v5 vs v4: nij padded to 4992 on host (39 full 128-row chunks, no E memset);
sim PSUM tiles are 2-bank quads [128,2,512] so one ACT exp covers 784 cols
(init overhead amortized: 838ns/4chunks vs 1024); warmup/qk/qvT share the
ps_vt pool and ctx halves share a single bank tile to fit PSUM in 8 banks;
q_proj is split b0-first so qk8(0) unblocks the exp stream sooner.
GPSIMD touches only SBUF (PSUM access is rejected by the BIR verifier) and
DoubleRow operand k-tile strides are padded to 16B alignment (4992/208).
"""

import os
import sys

sys.path.insert(0, "/opt/trn_rl_repo")

import numpy as np
import ml_dtypes

import concourse.bass as bass
import concourse.bacc as bacc
import concourse.mybir as mybir
import concourse.tile as tile
from concourse.bass_utils import run_bass_kernel_spmd
from concourse.masks import make_identity

F32 = mybir.dt.float32
BF16 = mybir.dt.bfloat16
FP8 = mybir.dt.float8e4

B_PER_CORE = 4
N_SUP = 25
C = 512
HW = 196
NIJ = N_SUP * HW          # 4900
DK = 128
NCH = 39                  # nij chunks of 128 (padded)
NIJP = NCH * 128          # 4992 padded nij: 16B-aligned for DoubleRow
HWP = 208                 # qk8 row pitch: 16B-aligned for DoubleRow
NQ = 10                   # sim quads per batch (last = 3 chunks)
VSCALE = 16.0             # host scales Wv by 16; ones column = 16 cancels it

DR = mybir.MatmulPerfMode.DoubleRow
EXP = mybir.ActivationFunctionType.Exp
MULT = mybir.AluOpType.mult
SUBTRACT = mybir.AluOpType.subtract


def build_bass():
    nc = bacc.Bacc(
        "TRN2", target_bir_lowering=False, debug=False, enable_asserts=False
    )
    s8_d = nc.dram_tensor(
        "s8", [B_PER_CORE, 2, 128, 2, NIJP], FP8, kind="ExternalInput"
    ).ap()
    wvT_d = nc.dram_tensor("wvT", [128, 4, DK], BF16, kind="ExternalInput").ap()
    q_d = nc.dram_tensor(
        "qbf", [128, B_PER_CORE, 4, HW], BF16, kind="ExternalInput"
    ).ap()
    # g8 = fp8(64 * Wqk^T Wqk) in DoubleRow lhsT layout [p, g, cc', t, m]
    g8_d = nc.dram_tensor(
        "g8", [128, 2, 4, 2, 128], FP8, kind="ExternalInput"
    ).ap()
    # q8 = fp8(Q) in DoubleRow rhs layout [p, b, g, t, hw]
    q8_d = nc.dram_tensor(
        "q8", [128, B_PER_CORE, 2, 2, HW], FP8, kind="ExternalInput"
    ).ap()
    wv8_d = nc.dram_tensor("wv8", [128, 2, 2, DK], FP8, kind="ExternalInput").ap()
    out_d = nc.dram_tensor(
        "out", [128, B_PER_CORE], F32, kind="ExternalOutput"
    ).ap()

    with tile.TileContext(nc) as tc:
        with (
            tc.tile_pool(name="const", bufs=1) as const,
            tc.tile_pool(name="s8p", bufs=8) as s8p,
            tc.tile_pool(name="etp", bufs=2) as etp,
            tc.tile_pool(name="vtp", bufs=3) as vtp,
            tc.tile_pool(name="qk8p", bufs=4) as qk8p,
            tc.tile_pool(name="small", bufs=8) as small,
            tc.tile_pool(name="ps_q", bufs=2, space="PSUM") as ps_q,
            tc.tile_pool(name="ps_vt", bufs=2, space="PSUM") as ps_vt,
            tc.tile_pool(name="ps_ctx", bufs=2, space="PSUM") as ps_ctx,
        ):
            # ---- input DMAs, ordered for fastest time-to-first-exp ----
            # Transfers FIFO-serialize at ~720B/ns in HWDGE issue order, so
            # the small q8/G8 pieces gating the qk chain go first on qSP
            # while wv8 interleaves from the qACT HWDGE queue.
            q8_sb = const.tile([128, B_PER_CORE, 2, 2, HW], FP8, tag="q8_sb")
            nc.sync.dma_start(out=q8_sb[:, 0], in_=q8_d[:, 0])
            g8_sb = const.tile([128, 2, 4, 2, 128], FP8, tag="g8_sb")
            nc.sync.dma_start(out=g8_sb, in_=g8_d)

            s8 = {}

            def s8_alloc(b):
                for g in range(2):
                    s8t = s8p.tile([128, 2, NIJP], FP8, tag="s8")
                    s8[(b, g)] = s8t

            def s8_piece(b, o, ln, eng=None):
                for g in range(2):
                    (eng or nc.sync).dma_start(
                        out=s8[(b, g)][:, :, o : o + ln],
                        in_=s8_d[b, g][:, :, o : o + ln],
                    )

            def s8_dma(b, pieces):
                s8_alloc(b)
                w = NIJP // pieces
                for i in range(pieces):
                    o = i * w
                    s8_piece(b, o, w if i < pieces - 1 else NIJP - o)

            s8_alloc(0)
            s8_alloc(1)
            wv8_sb = const.tile([128, 2, 2, DK], FP8, tag="wv8_sb")
            nc.scalar.dma_start(out=wv8_sb, in_=wv8_d)
            s8_piece(0, 0, 1280)
            s8_piece(0, 1280, 1280)
            wvT_sb = const.tile([128, 4, DK], BF16, tag="wvT_sb")
            nc.sync.dma_start(out=wvT_sb, in_=wvT_d)
            nc.sync.dma_start(out=q8_sb[:, 1:], in_=q8_d[:, 1:])
            q_sb = const.tile([128, B_PER_CORE, 4, HW], BF16, tag="q_sb")
            nc.sync.dma_start(out=q_sb[:, 0], in_=q_d[:, 0])
            s8_piece(0, 2560, 1280)
            s8_piece(0, 3840, 1152)
            nc.sync.dma_start(out=q_sb[:, 1:], in_=q_d[:, 1:])
            s8_piece(1, 0, 1248)
            s8_piece(1, 1248, 1248)
            s8_piece(1, 2496, 1248)
            s8_piece(1, 3744, 1248)

            # PE p-state warmup: wide matmuls on a zero tile start the PE
            # p-state ramp clock during the initial DMA wait so the real
            # prologue runs at or near full clock.
            warm_src = const.tile([128, 512], BF16, tag="warm_src")
            nc.vector.memset(warm_src, 0.0)
            # den ones vector (col 0: all rows; col 1: chunk-38 variant with
            # the 92 padded nij rows zeroed; 0-based memsets for the verifier)
            ones16 = const.tile([128, 2], BF16, tag="ones16")
            nc.vector.memset(ones16[:, 0:1], VSCALE)
            nc.vector.memset(ones16[:, 1:2], 0.0)
            nc.vector.memset(ones16[:36, 1:2], VSCALE)
            ident = const.tile([128, 128], F32, tag="ident")
            make_identity(nc, ident)
            for i in range(4):
                pw = ps_vt.tile([128, 512], F32, tag="ps_vt")
                nc.tensor.matmul(
                    pw,
                    lhsT=warm_src[:, 0:128],
                    rhs=warm_src,
                    start=True,
                    stop=True,
                )

            qv = {}

            def qv_prep(b):
                """qv[dv, hw] = Wv @ Q_b, natural orientation."""
                pt = ps_vt.tile([128, 512], F32, tag="ps_vt")
                for cc in range(4):
                    nc.tensor.matmul(
                        pt[:, 0:HW],
                        lhsT=wvT_sb[:, cc],
                        rhs=q_sb[:, b, cc],
                        start=(cc == 0),
                        stop=(cc == 3),
                    )
                qt = const.tile([128, HW], F32, tag=f"qv{b}")
                nc.vector.tensor_copy(qt, pt[:, 0:HW])
                qv[b] = qt

            # ---- per-batch stage generators (interleavable) ----
            et = {}
            vt1 = {}
            qk8 = {}

            def qk_prep(b):
                """qk = 64*(Wqk^T Wqk) @ Q via host G8, straight to fp8
                DoubleRow layout [128, 2(g), 2(t), 196]; the 64x scale is
                cancelled by the exp's 1/64 scale operand. One ps_q rotation
                turn; drained by ACT (a DVE drain here would stall the sim
                stream behind the DVE queue backlog)."""
                k8 = qk8p.tile([128, 2, 2, HWP], FP8, tag="qk8")
                pk = ps_q.tile([128, 2, 512], F32, tag="ps_q", name="pk")
                for go in range(2):
                    for t in range(2):
                        cc = 2 * go + t
                        for gi in range(2):
                            nc.tensor.matmul(
                                pk[:, go, t * HW : (t + 1) * HW],
                                lhsT=g8_sb[:, gi, cc],
                                rhs=q8_sb[:, b, gi],
                                start=(gi == 0),
                                stop=(gi == 1),
                                perf_mode=DR,
                            )
                nc.scalar.copy(
                    k8[:, :, :, 0:HW],
                    pk[:, :, 0 : 2 * HW].rearrange("p g (t hw) -> p g t hw", t=2),
                )
                qk8[b] = k8

            def sim_quad_gen(b):
                """Yield once per sim quad: 8 DR matmuls into a 2-bank PSUM
                tile + 1 (or 2, tail) quad-fused exp on ACT."""
                e = etp.tile([128, NCH * HW], BF16, tag="et")
                et[b] = e
                for jq in range(NQ):
                    chunks = range(4 * jq, min(4 * jq + 4, NCH))
                    ps = ps_q.tile([128, 2, 512], F32, tag="ps_q")
                    for ci, j in enumerate(chunks):
                        for g in range(2):
                            nc.tensor.matmul(
                                ps[
                                    :,
                                    ci // 2,
                                    (ci % 2) * HW : (ci % 2) * HW + HW,
                                ],
                                lhsT=s8[(b, g)][:, :, j * 128 : (j + 1) * 128],
                                rhs=qk8[b][:, g, :, 0:HW],
                                start=(g == 0),
                                stop=(g == 1),
                                perf_mode=DR,
                            )
                    o = 4 * jq * HW
                    if jq < NQ - 1:
                        nc.scalar.activation(
                            out=e[:, o : o + 4 * HW],
                            in_=ps[:, :, 0 : 2 * HW],
                            func=EXP,
                            scale=1.0 / 64.0,
                        )
                    else:
                        nc.scalar.activation(
                            out=e[:, o : o + 2 * HW],
                            in_=ps[:, 0, 0 : 2 * HW],
                            func=EXP,
                            scale=1.0 / 64.0,
                        )
                        nc.scalar.activation(
                            out=e[:, o + 2 * HW :],
                            in_=ps[:, 1, 0:HW],
                            func=EXP,
                            scale=1.0 / 64.0,
                        )
                    yield

            def vt_alloc(b):
                vt = vtp.tile([128, NCH * 128], BF16, tag="vt1")
                vt1[b] = vt

            def vt_gen(b, j_lo, j_hi):
                """Yield per V^T chunk: 2 DoubleRow matmuls into a [128,512]
                PSUM quad tile; one wide contiguous copy per quad."""
                for j0 in range(j_lo, j_hi, 4):
                    jn = min(4, j_hi - j0)
                    pq = ps_vt.tile([128, 512], F32, tag="ps_vt")
                    for ji in range(jn):
                        j = j0 + ji
                        for g in range(2):
                            nc.tensor.matmul(
                                pq[:, ji * 128 : (ji + 1) * 128],
                                lhsT=s8[(b, g)][:, :, j * 128 : (j + 1) * 128],
                                rhs=wv8_sb[:, g],
                                start=(g == 0),
                                stop=(g == 1),
                                perf_mode=DR,
                            )
                        yield
                    nc.vector.tensor_copy(
                        vt1[b][:, j0 * 128 : (j0 + jn) * 128],
                        pq[:, 0 : jn * 128],
                    )

            def ctx_alloc(b):
                # [*, 0:196] = ctx^T (dv x hw); [*, 200:202] = den halves
                return ps_ctx.tile([128, 512], F32, tag="ps_ctx", name="pcb")

            def pv_gen(b, pcb):
                """Yield per et chunk: ctx^T matmul (lhsT = V^T chunk, rhs =
                exp chunk, both hw halves at once) + 2 one-column den
                matmuls. den and ctx carry the same 16x scale (wv8 and the
                ones vector), so recip(den)*ctx cancels it exactly."""
                for j in range(NCH):
                    nc.tensor.matmul(
                        pcb[:, 0:HW],
                        lhsT=vt1[b][:, j * 128 : (j + 1) * 128],
                        rhs=et[b][:, j * HW : (j + 1) * HW],
                        start=(j == 0),
                        stop=(j == NCH - 1),
                    )
                    oc = 0 if j < NCH - 1 else 1
                    nc.tensor.matmul(
                        pcb[:128, 200:201],
                        lhsT=et[b][:, j * HW : j * HW + 128],
                        rhs=ones16[:, oc : oc + 1],
                        start=(j == 0),
                        stop=(j == NCH - 1),
                    )
                    nc.tensor.matmul(
                        pcb[:68, 201:202],
                        lhsT=et[b][:, j * HW + 128 : (j + 1) * HW],
                        rhs=ones16[:, oc : oc + 1],
                        start=(j == 0),
                        stop=(j == NCH - 1),
                    )
                    yield
                # epilogue: r = 1/den, transposed to the free dim via the PE,
                # then partial += sum((qv - ctx*r)^2) on the DVE
                r2 = small.tile([128, 2], F32, tag="r2")
                nc.vector.reciprocal(r2[:, 0:1], pcb[:, 200:201])
                nc.vector.reciprocal(r2[:68, 1:2], pcb[:68, 201:202])
                rr = ps_vt.tile([128, 512], F32, tag="ps_vt", name="rr")
                nc.tensor.matmul(
                    rr[0:1, 0:128],
                    lhsT=r2[:, 0:1],
                    rhs=ident[:, 0:128],
                    is_transpose=True,
                    start=True,
                    stop=True,
                )
                nc.tensor.matmul(
                    rr[0:1, 128:HW],
                    lhsT=r2[:68, 1:2],
                    rhs=ident[0:68, 0:68],
                    is_transpose=True,
                    start=True,
                    stop=True,
                )
                rs = small.tile([128, HW], F32, tag="rs")
                nc.vector.tensor_copy(rs[0:1, :], rr[0:1, 0:HW])
                rfull = small.tile([128, HW], F32, tag="rfull")
                nc.gpsimd.partition_broadcast(rfull, rs[0:1, :])
                t1 = small.tile([128, HW], F32, tag="t1")
                nc.vector.scalar_tensor_tensor(
                    t1,
                    pcb[:, 0:HW],
                    1.0,
                    rfull,
                    op0=MULT,
                    op1=MULT,
                )
                d = small.tile([128, HW], F32, tag="d")
                nc.vector.tensor_sub(d, t1, qv[b])
                d2 = small.tile([128, HW], F32, tag="d2")
                nc.vector.scalar_tensor_tensor(
                    d2,
                    d,
                    1.0,
                    d,
                    op0=MULT,
                    op1=MULT,
                    accum_out=partials[:, b : b + 1],
                )

            def drain(gen, n=None):
                if gen is None:
                    return None
                try:
                    if n is None:
                        while True:
                            next(gen)
                    else:
                        for _ in range(n):
                            next(gen)
                except StopIteration:
                    return None
                return gen

            # ---- schedule ----
            # PV(b) runs in-iteration, lagging its own exp stream by 2 slots;
            # its tail + ctx epilogue carry into iteration b+1.
            partials = const.tile([128, B_PER_CORE], F32, tag="partials")
            nc.vector.memset(partials, 0.0)
            qk_prep(0)
            vt_alloc(0)
            drain(vt_gen(0, 0, 8))      # keeps the PE busy until qk8 lands
            vt0_rest = vt_gen(0, 8, NCH)

            def sim_chain():
                for b in range(B_PER_CORE):
                    yield from sim_quad_gen(b)

            simg = sim_chain()
            carry = None        # pv tail from the previous batch
            vt_carry = None     # vt tail from the previous batch
            pvg = None

            for s in range(B_PER_CORE * NQ):
                b, jq = divmod(s, NQ)
                simg = drain(simg, 1)
                if jq == 0:
                    pcb = ctx_alloc(b)
                    pvg = pv_gen(b, pcb)
                    if b + 1 < B_PER_CORE:
                        vt_alloc(b + 1)
                        vtg = vt_gen(b + 1, 0, NCH)
                    else:
                        vtg = None
                    if b + 2 < B_PER_CORE:
                        s8_dma(b + 2, 4)
                if b == 0:
                    vt0_rest = drain(vt0_rest, 4)
                if jq < 2:
                    if vt_carry is not None:
                        vt_carry = drain(vt_carry, 3)
                    if carry is not None:
                        carry = drain(carry, 4)
                if b == 0:
                    if jq >= 5:
                        vtg = drain(vtg, 7)
                elif jq >= 1:
                    vtg = drain(vtg, 4)
                if jq >= 3:
                    pvg = drain(pvg, 5)
                if jq == 4 and b + 1 < B_PER_CORE:
                    qk_prep(b + 1)
                if jq == 6:
                    qv_prep(b)
                if jq == NQ - 1:
                    vt_carry = vtg
                    carry = pvg

            drain(carry)

            # final reduction happens on the host: DMA the partials matrix
            nc.sync.dma_start(out=out_d, in_=partials)

    nc.compile()
    return nc


_NC = None


def _prep_core(q, s, b0):
    """Host-side per-core input layouts (cast + transpose + zero-pad only)."""
    F8NP = ml_dtypes.float8_e4m3
    BFNP = ml_dtypes.bfloat16
    sb = s[b0 : b0 + B_PER_CORE]                      # [4, 25, 512, 196]
    s_c = sb.transpose(0, 2, 1, 3).reshape(B_PER_CORE, 2, 2, 128, NIJ)
    s8 = np.zeros((B_PER_CORE, 2, 128, 2, NIJP), dtype=F8NP)
    s8[:, :, :, :, :NIJ] = s_c.transpose(0, 1, 3, 2, 4).astype(F8NP)
    qb = q[b0 : b0 + B_PER_CORE]                      # [4, 512, 196]
    qbf = np.ascontiguousarray(
        qb.reshape(B_PER_CORE, 4, 128, HW).transpose(2, 0, 1, 3)
    ).astype(BFNP)                                    # [128, b, cc, hw]
    q8 = np.ascontiguousarray(
        qb.reshape(B_PER_CORE, 2, 2, 128, HW).transpose(3, 0, 1, 2, 4)
    ).astype(F8NP)                                    # [128, b, g, t, hw]
    return {"s8": s8, "qbf": qbf, "q8": q8}


def kernel(query_repr, supports_repr, W_qk, W_v):
    global _NC
    F8NP = ml_dtypes.float8_e4m3
    BFNP = ml_dtypes.bfloat16

    q = np.asarray(query_repr, dtype=np.float32).reshape(32, C, HW)
    s = np.asarray(supports_repr, dtype=np.float32).reshape(32, N_SUP, C, HW)
    wqk = np.asarray(W_qk, dtype=np.float32)
    wv = np.asarray(W_v, dtype=np.float32)

    wvT = np.ascontiguousarray(
        wv.T.reshape(4, 128, DK).transpose(1, 0, 2)
    ).astype(BFNP)
    wv8 = np.ascontiguousarray(
        (VSCALE * wv).T.reshape(2, 2, 128, DK).transpose(2, 0, 1, 3)
    ).astype(F8NP)
    # G8 = 64 * Wqk^T Wqk (symmetric), fp8 DoubleRow lhsT layout
    g = 64.0 * (wqk.T @ wqk)                          # [512, 512]
    g8 = np.ascontiguousarray(
        g.reshape(2, 2, 128, 4, 128).transpose(2, 0, 3, 1, 4)
    ).astype(F8NP)                                    # [p, g, cc', t, m]

    if _NC is None:
        _NC = build_bass()

    in_maps = []
    for core in range(8):
        m = _prep_core(q, s, core * B_PER_CORE)
        m.update({"wvT": wvT, "wv8": wv8, "g8": g8})
        in_maps.append(m)
    res = run_bass_kernel_spmd(
        _NC, in_maps, core_ids=list(range(8)),
        trace=bool(int(os.environ.get("KTRACE", "0"))),
    )
    total = sum(float(r["out"].astype(np.float64).sum()) for r in res.results)
    total = total / float(HW)
    kernel._last_results = res
    return np.asarray(total, dtype=np.float32)


# revision 62
# speedup vs baseline: 1.0884x; 1.0203x over previous
"""CrossTransformer kernel v5 for Trainium2 — fp8 sim-direct, quad-fused exp.

Per batch b (B=32 -> 4/core, N=25, C=512, H=W=14, DK=DV=128):
  qq = Wqk @ Q   (bf16)        qv = Wv @ Q  (bf16 -> f32)
  qk = Wqk^T @ qq  -> fp8 e4m3 DoubleRow layout [g][p][t][hw], c = g*256+t*128+p
  sim[nij,hw] = S^T @ qk       (fp8 DoubleRow; S is never projected to K)
  E = exp(sim) bf16            (ACT, quad-fused: one exp per 2 PSUM banks)
  V^T[nij,dv] = S^T @ (16*Wv)^T  (fp8 DoubleRow, direct transposed layout)
  ctx_raw[hw,129] = sum_j E_j^T @ [V^T_j | 16]   (ones=16 cancels the Wv scale)
  partial += sum((qv^T - num*recip(den))^2)

v5 vs v4: nij padded to 4992 on host (39 full 128-row chunks, no E memset);
sim PSUM tiles are 2-bank quads [128,2,512] so one ACT exp covers 784 cols
(init overhead amortized: 838ns/4chunks vs 1024); warmup/qk/qvT share the
ps_vt pool and ctx halves share a single bank tile to fit PSUM in 8 banks;
q_proj is split b0-first so qk8(0) unblocks the exp stream sooner.
GPSIMD touches only SBUF (PSUM access is rejected by the BIR verifier) and
DoubleRow operand k-tile strides are padded to 16B alignment (4992/208).
"""

import os
import sys

sys.path.insert(0, "/opt/trn_rl_repo")

import numpy as np
import ml_dtypes

import concourse.bass as bass
import concourse.bacc as bacc
import concourse.mybir as mybir
import concourse.tile as tile
from concourse.bass_utils import run_bass_kernel_spmd
from concourse.masks import make_identity

F32 = mybir.dt.float32
BF16 = mybir.dt.bfloat16
FP8 = mybir.dt.float8e4

B_PER_CORE = 4
N_SUP = 25
C = 512
HW = 196
NIJ = N_SUP * HW          # 4900
DK = 128
NCH = 39                  # nij chunks of 128 (padded)
NIJP = NCH * 128          # 4992 padded nij: 16B-aligned for DoubleRow
HWP = 208                 # qk8 row pitch: 16B-aligned for DoubleRow
NQ = 10                   # sim quads per batch (last = 3 chunks)
VSCALE = 16.0             # host scales Wv by 16; ones column = 16 cancels it

DR = mybir.MatmulPerfMode.DoubleRow
EXP = mybir.ActivationFunctionType.Exp
MULT = mybir.AluOpType.mult
SUBTRACT = mybir.AluOpType.subtract


def build_bass():
    nc = bacc.Bacc(
        "TRN2", target_bir_lowering=False, debug=False, enable_asserts=False
    )
    s8_d = nc.dram_tensor(
        "s8", [B_PER_CORE, 2, 128, 2, NIJP], FP8, kind="ExternalInput"
    ).ap()
    wvT_d = nc.dram_tensor("wvT", [128, 4, DK], BF16, kind="ExternalInput").ap()
    q_d = nc.dram_tensor(
        "qbf", [128, B_PER_CORE, 4, HW], BF16, kind="ExternalInput"
    ).ap()
    # g8 = fp8(64 * Wqk^T Wqk) in DoubleRow lhsT layout [p, g, cc', t, m]
    g8_d = nc.dram_tensor(
        "g8", [128, 2, 4, 2, 128], FP8, kind="ExternalInput"
    ).ap()
    # q8 = fp8(Q) in DoubleRow rhs layout [p, b, g, t, hw]
    q8_d = nc.dram_tensor(
        "q8", [128, B_PER_CORE, 2, 2, HW], FP8, kind="ExternalInput"
    ).ap()
    wv8_d = nc.dram_tensor("wv8", [128, 2, 2, DK], FP8, kind="ExternalInput").ap()
    out_d = nc.dram_tensor(
        "out", [128, B_PER_CORE], F32, kind="ExternalOutput"
    ).ap()

    with tile.TileContext(nc) as tc:
        with (
            tc.tile_pool(name="const", bufs=1) as const,
            tc.tile_pool(name="s8p", bufs=8) as s8p,
            tc.tile_pool(name="vtp", bufs=3) as vtp,
            tc.tile_pool(name="qk8p", bufs=4) as qk8p,
            tc.tile_pool(name="small", bufs=2) as small,
            tc.tile_pool(name="ps_q", bufs=2, space="PSUM") as ps_q,
            tc.tile_pool(name="ps_vt", bufs=2, space="PSUM") as ps_vt,
            tc.tile_pool(name="ps_ctx", bufs=2, space="PSUM") as ps_ctx,
        ):
            # ---- input DMAs, ordered for fastest time-to-first-exp ----
            # Transfers FIFO-serialize at ~720B/ns in HWDGE issue order, so
            # the small q8/G8 pieces gating the qk chain go first on qSP
            # while wv8 interleaves from the qACT HWDGE queue.
            q8_sb = const.tile([128, B_PER_CORE, 2, 2, HW], FP8, tag="q8_sb")
            nc.sync.dma_start(out=q8_sb[:, 0], in_=q8_d[:, 0])
            g8_sb = const.tile([128, 2, 4, 2, 128], FP8, tag="g8_sb")
            nc.sync.dma_start(out=g8_sb, in_=g8_d)

            s8 = {}

            def s8_alloc(b):
                for g in range(2):
                    s8t = s8p.tile([128, 2, NIJP], FP8, tag="s8")
                    s8[(b, g)] = s8t

            def s8_piece(b, o, ln, eng=None):
                for g in range(2):
                    (eng or nc.sync).dma_start(
                        out=s8[(b, g)][:, :, o : o + ln],
                        in_=s8_d[b, g][:, :, o : o + ln],
                    )

            def s8_dma(b, pieces):
                s8_alloc(b)
                w = NIJP // pieces
                for i in range(pieces):
                    o = i * w
                    s8_piece(b, o, w if i < pieces - 1 else NIJP - o)

            s8_alloc(0)
            s8_alloc(1)
            wv8_sb = const.tile([128, 2, 2, DK], FP8, tag="wv8_sb")
            nc.scalar.dma_start(out=wv8_sb, in_=wv8_d)
            s8_piece(0, 0, 1280)
            s8_piece(0, 1280, 1280)
            s8_piece(0, 2560, 1280)
            s8_piece(0, 3840, 1152)
            wvT_sb = const.tile([128, 4, DK], BF16, tag="wvT_sb")
            nc.sync.dma_start(out=wvT_sb, in_=wvT_d)
            nc.sync.dma_start(out=q8_sb[:, 1:], in_=q8_d[:, 1:])
            q_sb = const.tile([128, B_PER_CORE, 4, HW], BF16, tag="q_sb")
            nc.sync.dma_start(out=q_sb[:, 0], in_=q_d[:, 0])
            nc.sync.dma_start(out=q_sb[:, 1:], in_=q_d[:, 1:])
            s8_piece(1, 0, 1248)
            s8_piece(1, 1248, 1248)
            s8_piece(1, 2496, 1248)
            s8_piece(1, 3744, 1248)

            # PE p-state warmup: wide matmuls on a zero tile start the PE
            # p-state ramp clock during the initial DMA wait so the real
            # prologue runs at or near full clock.
            e_all = const.tile(
                [128, B_PER_CORE * NCH * HW], BF16, tag="e_all"
            )
            warm_src = const.tile([128, 512], BF16, tag="warm_src")
            nc.vector.memset(warm_src, 0.0)
            # den ones vector (col 0: all rows; col 1: chunk-38 variant with
            # the 92 padded nij rows zeroed; 0-based memsets for the verifier)
            ones16 = const.tile([128, 2], BF16, tag="ones16")
            nc.vector.memset(ones16[:, 0:1], VSCALE)
            nc.vector.memset(ones16[:, 1:2], 0.0)
            nc.vector.memset(ones16[:36, 1:2], VSCALE)
            ident = const.tile([128, 128], F32, tag="ident")
            make_identity(nc, ident)
            for i in range(6):
                pw = ps_vt.tile([128, 512], F32, tag="ps_vt")
                nc.tensor.matmul(
                    pw,
                    lhsT=warm_src[:, 0:128],
                    rhs=warm_src,
                    start=True,
                    stop=True,
                )

            qv = {}

            def qv_prep(b):
                """qv[dv, hw] = Wv @ Q_b, natural orientation."""
                pt = ps_vt.tile([128, 512], F32, tag="ps_vt")
                for cc in range(4):
                    nc.tensor.matmul(
                        pt[:, 0:HW],
                        lhsT=wvT_sb[:, cc],
                        rhs=q_sb[:, b, cc],
                        start=(cc == 0),
                        stop=(cc == 3),
                    )
                qt = const.tile([128, HW], F32, tag=f"qv{b}")
                nc.vector.tensor_copy(qt, pt[:, 0:HW])
                qv[b] = qt

            # ---- per-batch stage generators (interleavable) ----
            vt1 = {}
            qk8 = {}

            def qk_prep(b):
                """qk = 64*(Wqk^T Wqk) @ Q via host G8, straight to fp8
                DoubleRow layout [128, 2(g), 2(t), 196]; the 64x scale is
                cancelled by the exp's 1/64 scale operand. One ps_q rotation
                turn; drained by ACT (a DVE drain here would stall the sim
                stream behind the DVE queue backlog)."""
                k8 = qk8p.tile([128, 2, 2, HWP], FP8, tag="qk8")
                pk = ps_q.tile([128, 2, 512], F32, tag="ps_q", name="pk")
                for go in range(2):
                    for t in range(2):
                        cc = 2 * go + t
                        for gi in range(2):
                            nc.tensor.matmul(
                                pk[:, go, t * HW : (t + 1) * HW],
                                lhsT=g8_sb[:, gi, cc],
                                rhs=q8_sb[:, b, gi],
                                start=(gi == 0),
                                stop=(gi == 1),
                                perf_mode=DR,
                            )
                if b == 0:
                    # split drain: the first sim matmul (g=0) unblocks one
                    # ACT-copy earlier on the prologue critical path
                    for g in range(2):
                        nc.scalar.copy(
                            k8[:, g, :, 0:HW],
                            pk[:, g, 0 : 2 * HW].rearrange(
                                "p (t hw) -> p t hw", t=2
                            ),
                        )
                else:
                    nc.scalar.copy(
                        k8[:, :, :, 0:HW],
                        pk[:, :, 0 : 2 * HW].rearrange(
                            "p g (t hw) -> p g t hw", t=2
                        ),
                    )
                qk8[b] = k8

            def sim_quad_gen():
                """Yield once per sim quad: 8 DR matmuls into a 2-bank PSUM
                tile + 1 quad-fused exp on ACT. E for all 4 batches is one
                flat tile, so all 39 quads are full (they cross batch
                boundaries) and every exp is a uniform 784-column op."""
                for q in range(B_PER_CORE * NCH // 4):
                    ps = ps_q.tile([128, 2, 512], F32, tag="ps_q")
                    for ci in range(4):
                        c = 4 * q + ci
                        b, j = divmod(c, NCH)
                        for g in range(2):
                            nc.tensor.matmul(
                                ps[
                                    :,
                                    ci // 2,
                                    (ci % 2) * HW : (ci % 2) * HW + HW,
                                ],
                                lhsT=s8[(b, g)][:, :, j * 128 : (j + 1) * 128],
                                rhs=qk8[b][:, g, :, 0:HW],
                                start=(g == 0),
                                stop=(g == 1),
                                perf_mode=DR,
                            )
                    nc.scalar.activation(
                        out=e_all[:, 4 * q * HW : (4 * q + 4) * HW],
                        in_=ps[:, :, 0 : 2 * HW],
                        func=EXP,
                        scale=1.0 / 64.0,
                    )
                    yield

            def vt_alloc(b):
                vt = vtp.tile([128, NCH * 128], BF16, tag="vt1")
                vt1[b] = vt

            def vt_gen(b, j_lo, j_hi):
                """Yield per V^T chunk: 2 DoubleRow matmuls into a [128,512]
                PSUM quad tile; one wide contiguous copy per quad."""
                for j0 in range(j_lo, j_hi, 4):
                    jn = min(4, j_hi - j0)
                    pq = ps_vt.tile([128, 512], F32, tag="ps_vt")
                    for ji in range(jn):
                        j = j0 + ji
                        for g in range(2):
                            nc.tensor.matmul(
                                pq[:, ji * 128 : (ji + 1) * 128],
                                lhsT=s8[(b, g)][:, :, j * 128 : (j + 1) * 128],
                                rhs=wv8_sb[:, g],
                                start=(g == 0),
                                stop=(g == 1),
                                perf_mode=DR,
                            )
                        yield
                    nc.vector.tensor_copy(
                        vt1[b][:, j0 * 128 : (j0 + jn) * 128],
                        pq[:, 0 : jn * 128],
                    )

            def ctx_alloc(b):
                # [*, 0:196] = ctx^T (dv x hw); [*, 200:202] = den halves
                return ps_ctx.tile([128, 512], F32, tag="ps_ctx", name="pcb")

            def pv_gen(b, pcb):
                """Yield per et chunk: ctx^T matmul (lhsT = V^T chunk, rhs =
                exp chunk, both hw halves at once) + 2 one-column den
                matmuls. den and ctx carry the same 16x scale (wv8 and the
                ones vector), so recip(den)*ctx cancels it exactly."""
                for j in range(NCH):
                    nc.tensor.matmul(
                        pcb[:, 0:HW],
                        lhsT=vt1[b][:, j * 128 : (j + 1) * 128],
                        rhs=e_all[:, (b * NCH + j) * HW : (b * NCH + j + 1) * HW],
                        start=(j == 0),
                        stop=(j == NCH - 1),
                    )
                    oc = 0 if j < NCH - 1 else 1
                    nc.tensor.matmul(
                        pcb[:128, 200:201],
                        lhsT=e_all[:, (b * NCH + j) * HW : (b * NCH + j) * HW + 128],
                        rhs=ones16[:, oc : oc + 1],
                        start=(j == 0),
                        stop=(j == NCH - 1),
                    )
                    nc.tensor.matmul(
                        pcb[:68, 201:202],
                        lhsT=e_all[:, (b * NCH + j) * HW + 128 : (b * NCH + j + 1) * HW],
                        rhs=ones16[:, oc : oc + 1],
                        start=(j == 0),
                        stop=(j == NCH - 1),
                    )
                    yield
                # epilogue: r = 1/den, transposed to the free dim via the PE,
                # then partial += sum((qv - ctx*r)^2) on the DVE
                r2 = small.tile([128, 2], F32, tag="r2")
                nc.vector.reciprocal(r2[:, 0:1], pcb[:, 200:201])
                nc.vector.reciprocal(r2[:68, 1:2], pcb[:68, 201:202])
                rr = ps_vt.tile([128, 512], F32, tag="ps_vt", name="rr")
                nc.tensor.matmul(
                    rr[0:1, 0:128],
                    lhsT=r2[:, 0:1],
                    rhs=ident[:, 0:128],
                    is_transpose=True,
                    start=True,
                    stop=True,
                )
                nc.tensor.matmul(
                    rr[0:1, 128:HW],
                    lhsT=r2[:68, 1:2],
                    rhs=ident[0:68, 0:68],
                    is_transpose=True,
                    start=True,
                    stop=True,
                )
                rs = small.tile([128, HW], F32, tag="rs")
                nc.vector.tensor_copy(rs[0:1, :], rr[0:1, 0:HW])
                rfull = small.tile([128, HW], F32, tag="rfull")
                nc.gpsimd.partition_broadcast(rfull, rs[0:1, :])
                t1 = small.tile([128, HW], F32, tag="t1")
                nc.vector.scalar_tensor_tensor(
                    t1,
                    pcb[:, 0:HW],
                    1.0,
                    rfull,
                    op0=MULT,
                    op1=MULT,
                )
                d = small.tile([128, HW], F32, tag="d")
                nc.vector.tensor_sub(d, t1, qv[b])
                d2 = small.tile([128, HW], F32, tag="d2")
                nc.vector.scalar_tensor_tensor(
                    d2,
                    d,
                    1.0,
                    d,
                    op0=MULT,
                    op1=MULT,
                    accum_out=partials[:, b : b + 1],
                )

            def drain(gen, n=None):
                if gen is None:
                    return None
                try:
                    if n is None:
                        while True:
                            next(gen)
                    else:
                        for _ in range(n):
                            next(gen)
                except StopIteration:
                    return None
                return gen

            # ---- schedule ----
            # PV(b) runs in-iteration, lagging its own exp stream by 2 slots;
            # its tail + ctx epilogue carry into iteration b+1.
            partials = const.tile([128, B_PER_CORE], F32, tag="partials")
            nc.vector.memset(partials, 0.0)
            qk_prep(0)
            vt_alloc(0)
            drain(vt_gen(0, 0, 8))      # keeps the PE busy until qk8 lands
            vt0_rest = vt_gen(0, 8, NCH)

            simg = sim_quad_gen()
            carry = None        # pv tail from the previous batch
            vt_carry = None     # vt tail from the previous batch
            pvg = None

            for s in range(B_PER_CORE * NQ):
                b, jq = divmod(s, NQ)
                simg = drain(simg, 1)
                if jq == 0:
                    pcb = ctx_alloc(b)
                    pvg = pv_gen(b, pcb)
                    if b + 1 < B_PER_CORE:
                        vt_alloc(b + 1)
                        vtg = vt_gen(b + 1, 0, NCH)
                    else:
                        vtg = None
                    if b + 2 < B_PER_CORE:
                        s8_dma(b + 2, 4)
                if b == 0:
                    vt0_rest = drain(vt0_rest, 4)
                if jq < 2:
                    if vt_carry is not None:
                        vt_carry = drain(vt_carry, 3)
                    if carry is not None:
                        carry = drain(carry, 4)
                if b == 0:
                    if jq >= 5:
                        vtg = drain(vtg, 7)
                elif jq >= 1:
                    vtg = drain(vtg, 4)
                if jq >= 3:
                    pvg = drain(pvg, 5)
                if jq == 4 and b + 1 < B_PER_CORE:
                    qk_prep(b + 1)
                if jq == 6:
                    qv_prep(b)
                if jq == NQ - 1:
                    vt_carry = vtg
                    carry = pvg

            drain(carry)

            # final reduction happens on the host: DMA the partials matrix
            nc.sync.dma_start(out=out_d, in_=partials)

    nc.compile()
    return nc


_NC = None


def _prep_core(q, s, b0):
    """Host-side per-core input layouts (cast + transpose + zero-pad only)."""
    F8NP = ml_dtypes.float8_e4m3
    BFNP = ml_dtypes.bfloat16
    sb = s[b0 : b0 + B_PER_CORE]                      # [4, 25, 512, 196]
    s_c = sb.transpose(0, 2, 1, 3).reshape(B_PER_CORE, 2, 2, 128, NIJ)
    s8 = np.zeros((B_PER_CORE, 2, 128, 2, NIJP), dtype=F8NP)
    s8[:, :, :, :, :NIJ] = s_c.transpose(0, 1, 3, 2, 4).astype(F8NP)
    qb = q[b0 : b0 + B_PER_CORE]                      # [4, 512, 196]
    qbf = np.ascontiguousarray(
        qb.reshape(B_PER_CORE, 4, 128, HW).transpose(2, 0, 1, 3)
    ).astype(BFNP)                                    # [128, b, cc, hw]
    q8 = np.ascontiguousarray(
        qb.reshape(B_PER_CORE, 2, 2, 128, HW).transpose(3, 0, 1, 2, 4)
    ).astype(F8NP)                                    # [128, b, g, t, hw]
    return {"s8": s8, "qbf": qbf, "q8": q8}


def kernel(query_repr, supports_repr, W_qk, W_v):
    global _NC
    F8NP = ml_dtypes.float8_e4m3
    BFNP = ml_dtypes.bfloat16

    q = np.asarray(query_repr, dtype=np.float32).reshape(32, C, HW)
    s = np.asarray(supports_repr, dtype=np.float32).reshape(32, N_SUP, C, HW)
    wqk = np.asarray(W_qk, dtype=np.float32)
    wv = np.asarray(W_v, dtype=np.float32)

    wvT = np.ascontiguousarray(
        wv.T.reshape(4, 128, DK).transpose(1, 0, 2)
    ).astype(BFNP)
    wv8 = np.ascontiguousarray(
        (VSCALE * wv).T.reshape(2, 2, 128, DK).transpose(2, 0, 1, 3)
    ).astype(F8NP)
    # G8 = 64 * Wqk^T Wqk (symmetric), fp8 DoubleRow lhsT layout
    g = 64.0 * (wqk.T @ wqk)                          # [512, 512]
    g8 = np.ascontiguousarray(
        g.reshape(2, 2, 128, 4, 128).transpose(2, 0, 3, 1, 4)
    ).astype(F8NP)                                    # [p, g, cc', t, m]

    if _NC is None:
        _NC = build_bass()

    in_maps = []
    for core in range(8):
        m = _prep_core(q, s, core * B_PER_CORE)
        m.update({"wvT": wvT, "wv8": wv8, "g8": g8})
        in_maps.append(m)
    res = run_bass_kernel_spmd(
        _NC, in_maps, core_ids=list(range(8)),
        trace=bool(int(os.environ.get("KTRACE", "0"))),
    )
    total = sum(float(r["out"].astype(np.float64).sum()) for r in res.results)
    total = total / float(HW)
    kernel._last_results = res
    return np.asarray(total, dtype=np.float32)


# revision 71
# speedup vs baseline: 1.1033x; 1.0137x over previous
"""CrossTransformer kernel v5 for Trainium2 — fp8 sim-direct, quad-fused exp.

Per batch b (B=32 -> 4/core, N=25, C=512, H=W=14, DK=DV=128):
  qq = Wqk @ Q   (bf16)        qv = Wv @ Q  (bf16 -> f32)
  qk = Wqk^T @ qq  -> fp8 e4m3 DoubleRow layout [g][p][t][hw], c = g*256+t*128+p
  sim[nij,hw] = S^T @ qk       (fp8 DoubleRow; S is never projected to K)
  E = exp(sim) bf16            (ACT, quad-fused: one exp per 2 PSUM banks)
  V^T[nij,dv] = S^T @ (16*Wv)^T  (fp8 DoubleRow, direct transposed layout)
  ctx_raw[hw,129] = sum_j E_j^T @ [V^T_j | 16]   (ones=16 cancels the Wv scale)
  partial += sum((qv^T - num*recip(den))^2)

v5 vs v4: nij padded to 4992 on host (39 full 128-row chunks, no E memset);
sim PSUM tiles are 2-bank quads [128,2,512] so one ACT exp covers 784 cols
(init overhead amortized: 838ns/4chunks vs 1024); warmup/qk/qvT share the
ps_vt pool and ctx halves share a single bank tile to fit PSUM in 8 banks;
q_proj is split b0-first so qk8(0) unblocks the exp stream sooner.
GPSIMD touches only SBUF (PSUM access is rejected by the BIR verifier) and
DoubleRow operand k-tile strides are padded to 16B alignment (4992/208).
"""

import os
import sys

sys.path.insert(0, "/opt/trn_rl_repo")

import numpy as np
import ml_dtypes

import concourse.bass as bass
import concourse.bacc as bacc
import concourse.mybir as mybir
import concourse.tile as tile
from concourse.bass_utils import run_bass_kernel_spmd
from concourse.masks import make_identity

F32 = mybir.dt.float32
BF16 = mybir.dt.bfloat16
FP8 = mybir.dt.float8e4

B_PER_CORE = 4
N_SUP = 25
C = 512
HW = 196
NIJ = N_SUP * HW          # 4900
DK = 128
NCH = 39                  # nij chunks of 128 (padded)
NIJP = NCH * 128          # 4992 padded nij: 16B-aligned for DoubleRow
HWP = 208                 # qk8 row pitch: 16B-aligned for DoubleRow
NQ = 10                   # sim quads per batch (last = 3 chunks)
VSCALE = 16.0             # host scales Wv by 16; ones column = 16 cancels it

DR = mybir.MatmulPerfMode.DoubleRow
EXP = mybir.ActivationFunctionType.Exp
MULT = mybir.AluOpType.mult
SUBTRACT = mybir.AluOpType.subtract


def build_bass():
    nc = bacc.Bacc(
        "TRN2", target_bir_lowering=False, debug=False, enable_asserts=False
    )
    s8_d = nc.dram_tensor(
        "s8", [B_PER_CORE, 2, 128, 2, NIJP], FP8, kind="ExternalInput"
    ).ap()
    wvT_d = nc.dram_tensor("wvT", [128, 4, DK], BF16, kind="ExternalInput").ap()
    q_d = nc.dram_tensor(
        "qbf", [128, B_PER_CORE, 4, HW], BF16, kind="ExternalInput"
    ).ap()
    # g8 = fp8(64 * Wqk^T Wqk) in DoubleRow lhsT layout [p, g, cc', t, m]
    g8_d = nc.dram_tensor(
        "g8", [128, 2, 4, 2, 128], FP8, kind="ExternalInput"
    ).ap()
    # q8 = fp8(Q) in DoubleRow rhs layout [p, b, g, t, hw]
    q8_d = nc.dram_tensor(
        "q8", [128, B_PER_CORE, 2, 2, HW], FP8, kind="ExternalInput"
    ).ap()
    wv8_d = nc.dram_tensor("wv8", [128, 2, 2, DK], FP8, kind="ExternalInput").ap()
    out_d = nc.dram_tensor(
        "out", [128, B_PER_CORE], F32, kind="ExternalOutput"
    ).ap()

    with tile.TileContext(nc) as tc:
        with (
            tc.tile_pool(name="const", bufs=1) as const,
            tc.tile_pool(name="s8p", bufs=8) as s8p,
            tc.tile_pool(name="vtp", bufs=3) as vtp,
            tc.tile_pool(name="qk8p", bufs=4) as qk8p,
            tc.tile_pool(name="small", bufs=2) as small,
            tc.tile_pool(name="ps_q", bufs=2, space="PSUM") as ps_q,
            tc.tile_pool(name="ps_vt", bufs=2, space="PSUM") as ps_vt,
            tc.tile_pool(name="ps_ctx", bufs=2, space="PSUM") as ps_ctx,
        ):
            # ---- input DMAs, ordered for fastest time-to-first-exp ----
            # Transfers FIFO-serialize at ~720B/ns in HWDGE issue order, so
            # the small q8/G8 pieces gating the qk chain go first on qSP
            # while wv8 interleaves from the qACT HWDGE queue.
            q8_sb = const.tile([128, B_PER_CORE, 2, 2, HW], FP8, tag="q8_sb")
            nc.sync.dma_start(out=q8_sb[:, 0], in_=q8_d[:, 0])
            g8_sb = const.tile([128, 2, 4, 2, 128], FP8, tag="g8_sb")
            nc.sync.dma_start(out=g8_sb, in_=g8_d)

            s8 = {}

            def s8_alloc(b):
                for g in range(2):
                    s8t = s8p.tile([128, 2, NIJP], FP8, tag="s8")
                    s8[(b, g)] = s8t

            def s8_piece(b, o, ln, eng=None):
                for g in range(2):
                    (eng or nc.sync).dma_start(
                        out=s8[(b, g)][:, :, o : o + ln],
                        in_=s8_d[b, g][:, :, o : o + ln],
                    )

            def s8_dma(b, pieces):
                s8_alloc(b)
                w = NIJP // pieces
                for i in range(pieces):
                    o = i * w
                    s8_piece(b, o, w if i < pieces - 1 else NIJP - o)

            s8_alloc(0)
            s8_alloc(1)
            wv8_sb = const.tile([128, 2, 2, DK], FP8, tag="wv8_sb")
            nc.scalar.dma_start(out=wv8_sb, in_=wv8_d)
            s8_piece(0, 0, 1280)
            s8_piece(0, 1280, 1280)
            s8_piece(0, 2560, 1280)
            s8_piece(0, 3840, 1152)
            wvT_sb = const.tile([128, 4, DK], BF16, tag="wvT_sb")
            nc.sync.dma_start(out=wvT_sb, in_=wvT_d)
            nc.sync.dma_start(out=q8_sb[:, 1:], in_=q8_d[:, 1:])
            q_sb = const.tile([128, B_PER_CORE, 4, HW], BF16, tag="q_sb")
            nc.sync.dma_start(out=q_sb[:, 0], in_=q_d[:, 0])
            s8_piece(1, 0, 1248)
            s8_piece(1, 1248, 1248)
            nc.sync.dma_start(out=q_sb[:, 1:], in_=q_d[:, 1:])
            s8_piece(1, 2496, 1248)
            s8_piece(1, 3744, 1248)

            # PE p-state warmup: wide matmuls on a zero tile start the PE
            # p-state ramp clock during the initial DMA wait so the real
            # prologue runs at or near full clock.
            e_all = const.tile(
                [128, B_PER_CORE * NCH * HW], BF16, tag="e_all"
            )
            warm_src = const.tile([128, 512], BF16, tag="warm_src")
            nc.vector.memset(warm_src, 0.0)
            # den ones vector (col 0: all rows; col 1: chunk-38 variant with
            # the 92 padded nij rows zeroed; 0-based memsets for the verifier)
            ones16 = const.tile([128, 2], BF16, tag="ones16")
            nc.vector.memset(ones16[:, 0:1], VSCALE)
            nc.vector.memset(ones16[:, 1:2], 0.0)
            nc.vector.memset(ones16[:36, 1:2], VSCALE)
            ident = const.tile([128, 128], F32, tag="ident")
            make_identity(nc, ident)
            for i in range(5):
                pw = ps_vt.tile([128, 512], F32, tag="ps_vt")
                nc.tensor.matmul(
                    pw,
                    lhsT=warm_src[:, 0:128],
                    rhs=warm_src,
                    start=True,
                    stop=True,
                )

            qv = {}

            def qv_prep(b):
                """qv[dv, hw] = Wv @ Q_b, natural orientation."""
                pt = ps_vt.tile([128, 512], F32, tag="ps_vt")
                for cc in range(4):
                    nc.tensor.matmul(
                        pt[:, 0:HW],
                        lhsT=wvT_sb[:, cc],
                        rhs=q_sb[:, b, cc],
                        start=(cc == 0),
                        stop=(cc == 3),
                    )
                qt = const.tile([128, HW], F32, tag=f"qv{b}")
                nc.vector.tensor_copy(qt, pt[:, 0:HW])
                qv[b] = qt

            # ---- per-batch stage generators (interleavable) ----
            vt1 = {}
            qk8 = {}

            def qk_prep(b):
                """qk = 64*(Wqk^T Wqk) @ Q via host G8, straight to fp8
                DoubleRow layout [128, 2(g), 2(t), 196]; the 64x scale is
                cancelled by the exp's 1/64 scale operand. All four batches
                run in the prologue: b==0 through the ps_q pool with split
                ACT drains (fastest path to the first exp), b>=1 through
                2-wave 1-bank ps_vt tiles with DVE drains so neither the
                exp stream nor the sim-quad PSUM rotation ever waits."""
                k8 = qk8p.tile([128, 2, 2, HWP], FP8, tag="qk8")
                if b == 0:
                    pk = ps_q.tile([128, 2, 512], F32, tag="ps_q", name="pk")
                    for go in range(2):
                        for t in range(2):
                            cc = 2 * go + t
                            for gi in range(2):
                                nc.tensor.matmul(
                                    pk[:, go, t * HW : (t + 1) * HW],
                                    lhsT=g8_sb[:, gi, cc],
                                    rhs=q8_sb[:, b, gi],
                                    start=(gi == 0),
                                    stop=(gi == 1),
                                    perf_mode=DR,
                                )
                    for g in range(2):
                        nc.scalar.copy(
                            k8[:, g, :, 0:HW],
                            pk[:, g, 0 : 2 * HW].rearrange(
                                "p (t hw) -> p t hw", t=2
                            ),
                        )
                else:
                    pk = ps_q.tile([128, 2, 512], F32, tag="ps_q", name="pk")
                    for go in range(2):
                        for t in range(2):
                            cc = 2 * go + t
                            for gi in range(2):
                                nc.tensor.matmul(
                                    pk[:, go, t * HW : (t + 1) * HW],
                                    lhsT=g8_sb[:, gi, cc],
                                    rhs=q8_sb[:, b, gi],
                                    start=(gi == 0),
                                    stop=(gi == 1),
                                    perf_mode=DR,
                                )
                    nc.scalar.copy(
                        k8[:, :, :, 0:HW],
                        pk[:, :, 0 : 2 * HW].rearrange(
                            "p g (t hw) -> p g t hw", t=2
                        ),
                    )
                qk8[b] = k8

            def sim_quad_gen():
                """Yield once per sim quad: 8 DR matmuls into a 2-bank PSUM
                tile + 1 quad-fused exp on ACT. E for all 4 batches is one
                flat tile, so all 39 quads are full (they cross batch
                boundaries) and every exp is a uniform 784-column op."""
                for q in range(B_PER_CORE * NCH // 4):
                    ps = ps_q.tile([128, 2, 512], F32, tag="ps_q")
                    for ci in range(4):
                        c = 4 * q + ci
                        b, j = divmod(c, NCH)
                        for g in range(2):
                            nc.tensor.matmul(
                                ps[
                                    :,
                                    ci // 2,
                                    (ci % 2) * HW : (ci % 2) * HW + HW,
                                ],
                                lhsT=s8[(b, g)][:, :, j * 128 : (j + 1) * 128],
                                rhs=qk8[b][:, g, :, 0:HW],
                                start=(g == 0),
                                stop=(g == 1),
                                perf_mode=DR,
                            )
                    nc.scalar.activation(
                        out=e_all[:, 4 * q * HW : (4 * q + 4) * HW],
                        in_=ps[:, :, 0 : 2 * HW],
                        func=EXP,
                        scale=1.0 / 64.0,
                    )
                    yield

            def vt_alloc(b):
                vt = vtp.tile([128, NCH * 128], BF16, tag="vt1")
                vt1[b] = vt

            def vt_gen(b, j_lo, j_hi):
                """Yield per V^T chunk: 2 DoubleRow matmuls into a [128,512]
                PSUM quad tile; one wide contiguous copy per quad."""
                for j0 in range(j_lo, j_hi, 4):
                    jn = min(4, j_hi - j0)
                    pq = ps_vt.tile([128, 512], F32, tag="ps_vt")
                    for ji in range(jn):
                        j = j0 + ji
                        for g in range(2):
                            nc.tensor.matmul(
                                pq[:, ji * 128 : (ji + 1) * 128],
                                lhsT=s8[(b, g)][:, :, j * 128 : (j + 1) * 128],
                                rhs=wv8_sb[:, g],
                                start=(g == 0),
                                stop=(g == 1),
                                perf_mode=DR,
                            )
                        yield
                    nc.vector.tensor_copy(
                        vt1[b][:, j0 * 128 : (j0 + jn) * 128],
                        pq[:, 0 : jn * 128],
                    )

            def ctx_alloc(b):
                # [*, 0:196] = ctx^T (dv x hw); [*, 200:202] = den halves
                return ps_ctx.tile([128, 512], F32, tag="ps_ctx", name="pcb")

            def pv_gen(b, pcb):
                """Yield per et chunk: ctx^T matmul (lhsT = V^T chunk, rhs =
                exp chunk, both hw halves at once) + 2 one-column den
                matmuls. den and ctx carry the same 16x scale (wv8 and the
                ones vector), so recip(den)*ctx cancels it exactly."""
                for j in range(NCH):
                    nc.tensor.matmul(
                        pcb[:, 0:HW],
                        lhsT=vt1[b][:, j * 128 : (j + 1) * 128],
                        rhs=e_all[:, (b * NCH + j) * HW : (b * NCH + j + 1) * HW],
                        start=(j == 0),
                        stop=(j == NCH - 1),
                    )
                    oc = 0 if j < NCH - 1 else 1
                    nc.tensor.matmul(
                        pcb[:128, 200:201],
                        lhsT=e_all[:, (b * NCH + j) * HW : (b * NCH + j) * HW + 128],
                        rhs=ones16[:, oc : oc + 1],
                        start=(j == 0),
                        stop=(j == NCH - 1),
                    )
                    nc.tensor.matmul(
                        pcb[:68, 201:202],
                        lhsT=e_all[:, (b * NCH + j) * HW + 128 : (b * NCH + j + 1) * HW],
                        rhs=ones16[:, oc : oc + 1],
                        start=(j == 0),
                        stop=(j == NCH - 1),
                    )
                    yield
                # epilogue: r = 1/den, transposed to the free dim via the PE,
                # then partial += sum((qv - ctx*r)^2) on the DVE
                r2 = small.tile([128, 2], F32, tag="r2")
                nc.vector.reciprocal(r2[:, 0:1], pcb[:, 200:201])
                nc.vector.reciprocal(r2[:68, 1:2], pcb[:68, 201:202])
                rr = ps_vt.tile([128, 512], F32, tag="ps_vt", name="rr")
                nc.tensor.matmul(
                    rr[0:1, 0:128],
                    lhsT=r2[:, 0:1],
                    rhs=ident[:, 0:128],
                    is_transpose=True,
                    start=True,
                    stop=True,
                )
                nc.tensor.matmul(
                    rr[0:1, 128:HW],
                    lhsT=r2[:68, 1:2],
                    rhs=ident[0:68, 0:68],
                    is_transpose=True,
                    start=True,
                    stop=True,
                )
                rs = small.tile([128, HW], F32, tag="rs")
                nc.vector.tensor_copy(rs[0:1, :], rr[0:1, 0:HW])
                rfull = small.tile([128, HW], F32, tag="rfull")
                nc.gpsimd.partition_broadcast(rfull, rs[0:1, :])
                t1 = small.tile([128, HW], F32, tag="t1")
                nc.vector.scalar_tensor_tensor(
                    t1,
                    pcb[:, 0:HW],
                    1.0,
                    rfull,
                    op0=MULT,
                    op1=MULT,
                )
                d = small.tile([128, HW], F32, tag="d")
                nc.vector.tensor_sub(d, t1, qv[b])
                d2 = small.tile([128, HW], F32, tag="d2")
                nc.vector.scalar_tensor_tensor(
                    d2,
                    d,
                    1.0,
                    d,
                    op0=MULT,
                    op1=MULT,
                    accum_out=partials[:, b : b + 1],
                )

            def drain(gen, n=None):
                if gen is None:
                    return None
                try:
                    if n is None:
                        while True:
                            next(gen)
                    else:
                        for _ in range(n):
                            next(gen)
                except StopIteration:
                    return None
                return gen

            # ---- schedule ----
            # PV(b) runs in-iteration, lagging its own exp stream by 2 slots;
            # its tail + ctx epilogue carry into iteration b+1.
            partials = const.tile([128, B_PER_CORE], F32, tag="partials")
            nc.vector.memset(partials, 0.0)
            qk_prep(0)
            vt_alloc(0)
            drain(vt_gen(0, 0, 8))      # keeps the PE busy until qk8 lands
            vt0_rest = vt_gen(0, 8, NCH)

            simg = sim_quad_gen()
            carry = None        # pv tail from the previous batch
            vt_carry = None     # vt tail from the previous batch
            pvg = None

            for s in range(B_PER_CORE * NQ):
                b, jq = divmod(s, NQ)
                simg = drain(simg, 1)
                if jq == 0:
                    pcb = ctx_alloc(b)
                    pvg = pv_gen(b, pcb)
                    if b + 1 < B_PER_CORE:
                        vt_alloc(b + 1)
                        vtg = vt_gen(b + 1, 0, NCH)
                    else:
                        vtg = None
                    if b + 2 < B_PER_CORE:
                        s8_dma(b + 2, 4)
                if b == 0:
                    vt0_rest = drain(vt0_rest, 4)
                if jq == 4 and b + 1 < B_PER_CORE:
                    qk_prep(b + 1)
                if jq < 2:
                    if vt_carry is not None:
                        vt_carry = drain(vt_carry, 3)
                    if carry is not None:
                        carry = drain(carry, 4)
                if b == 0:
                    if jq >= 5:
                        vtg = drain(vtg, 7)
                elif jq >= 1:
                    vtg = drain(vtg, 4)
                if jq >= 3:
                    pvg = drain(pvg, 5)
                if jq == 7:
                    qv_prep(b)
                if jq == NQ - 1:
                    vt_carry = vtg
                    carry = pvg

            drain(carry)

            # final reduction happens on the host: DMA the partials matrix
            nc.sync.dma_start(out=out_d, in_=partials)

    nc.compile()
    return nc


_NC = None


def _prep_core(q, s, b0):
    """Host-side per-core input layouts (cast + transpose + zero-pad only)."""
    F8NP = ml_dtypes.float8_e4m3
    BFNP = ml_dtypes.bfloat16
    sb = s[b0 : b0 + B_PER_CORE]                      # [4, 25, 512, 196]
    s_c = sb.transpose(0, 2, 1, 3).reshape(B_PER_CORE, 2, 2, 128, NIJ)
    s8 = np.zeros((B_PER_CORE, 2, 128, 2, NIJP), dtype=F8NP)
    s8[:, :, :, :, :NIJ] = s_c.transpose(0, 1, 3, 2, 4).astype(F8NP)
    qb = q[b0 : b0 + B_PER_CORE]                      # [4, 512, 196]
    qbf = np.ascontiguousarray(
        qb.reshape(B_PER_CORE, 4, 128, HW).transpose(2, 0, 1, 3)
    ).astype(BFNP)                                    # [128, b, cc, hw]
    q8 = np.ascontiguousarray(
        qb.reshape(B_PER_CORE, 2, 2, 128, HW).transpose(3, 0, 1, 2, 4)
    ).astype(F8NP)                                    # [128, b, g, t, hw]
    return {"s8": s8, "qbf": qbf, "q8": q8}


def kernel(query_repr, supports_repr, W_qk, W_v):
    global _NC
    F8NP = ml_dtypes.float8_e4m3
    BFNP = ml_dtypes.bfloat16

    q = np.asarray(query_repr, dtype=np.float32).reshape(32, C, HW)
    s = np.asarray(supports_repr, dtype=np.float32).reshape(32, N_SUP, C, HW)
    wqk = np.asarray(W_qk, dtype=np.float32)
    wv = np.asarray(W_v, dtype=np.float32)

    wvT = np.ascontiguousarray(
        wv.T.reshape(4, 128, DK).transpose(1, 0, 2)
    ).astype(BFNP)
    wv8 = np.ascontiguousarray(
        (VSCALE * wv).T.reshape(2, 2, 128, DK).transpose(2, 0, 1, 3)
    ).astype(F8NP)
    # G8 = 64 * Wqk^T Wqk (symmetric), fp8 DoubleRow lhsT layout
    g = 64.0 * (wqk.T @ wqk)                          # [512, 512]
    g8 = np.ascontiguousarray(
        g.reshape(2, 2, 128, 4, 128).transpose(2, 0, 3, 1, 4)
    ).astype(F8NP)                                    # [p, g, cc', t, m]

    if _NC is None:
        _NC = build_bass()

    in_maps = []
    for core in range(8):
        m = _prep_core(q, s, core * B_PER_CORE)
        m.update({"wvT": wvT, "wv8": wv8, "g8": g8})
        in_maps.append(m)
    res = run_bass_kernel_spmd(
        _NC, in_maps, core_ids=list(range(8)),
        trace=bool(int(os.environ.get("KTRACE", "0"))),
    )
    total = sum(float(r["out"].astype(np.float64).sum()) for r in res.results)
    total = total / float(HW)
    kernel._last_results = res
    return np.asarray(total, dtype=np.float32)


# revision 76
# speedup vs baseline: 1.1061x; 1.0025x over previous
"""CrossTransformer kernel v5 for Trainium2 — fp8 sim-direct, quad-fused exp.

Per batch b (B=32 -> 4/core, N=25, C=512, H=W=14, DK=DV=128):
  qq = Wqk @ Q   (bf16)        qv = Wv @ Q  (bf16 -> f32)
  qk = Wqk^T @ qq  -> fp8 e4m3 DoubleRow layout [g][p][t][hw], c = g*256+t*128+p
  sim[nij,hw] = S^T @ qk       (fp8 DoubleRow; S is never projected to K)
  E = exp(sim) bf16            (ACT, quad-fused: one exp per 2 PSUM banks)
  V^T[nij,dv] = S^T @ (16*Wv)^T  (fp8 DoubleRow, direct transposed layout)
  ctx_raw[hw,129] = sum_j E_j^T @ [V^T_j | 16]   (ones=16 cancels the Wv scale)
  partial += sum((qv^T - num*recip(den))^2)

v5 vs v4: nij padded to 4992 on host (39 full 128-row chunks, no E memset);
sim PSUM tiles are 2-bank quads [128,2,512] so one ACT exp covers 784 cols
(init overhead amortized: 838ns/4chunks vs 1024); warmup/qk/qvT share the
ps_vt pool and ctx halves share a single bank tile to fit PSUM in 8 banks;
q_proj is split b0-first so qk8(0) unblocks the exp stream sooner.
GPSIMD touches only SBUF (PSUM access is rejected by the BIR verifier) and
DoubleRow operand k-tile strides are padded to 16B alignment (4992/208).
"""

import os
import sys

sys.path.insert(0, "/opt/trn_rl_repo")

import numpy as np
import ml_dtypes

import concourse.bass as bass
import concourse.bacc as bacc
import concourse.mybir as mybir
import concourse.tile as tile
from concourse.bass_utils import run_bass_kernel_spmd
from concourse.masks import make_identity

F32 = mybir.dt.float32
BF16 = mybir.dt.bfloat16
FP8 = mybir.dt.float8e4

B_PER_CORE = 4
N_SUP = 25
C = 512
HW = 196
NIJ = N_SUP * HW          # 4900
DK = 128
NCH = 39                  # nij chunks of 128 (padded)
NIJP = NCH * 128          # 4992 padded nij: 16B-aligned for DoubleRow
HWP = 208                 # qk8 row pitch: 16B-aligned for DoubleRow
NQ = 10                   # sim quads per batch (last = 3 chunks)
VSCALE = 16.0             # host scales Wv by 16; ones column = 16 cancels it

DR = mybir.MatmulPerfMode.DoubleRow
EXP = mybir.ActivationFunctionType.Exp
MULT = mybir.AluOpType.mult
SUBTRACT = mybir.AluOpType.subtract


def build_bass():
    nc = bacc.Bacc(
        "TRN2", target_bir_lowering=False, debug=False, enable_asserts=False
    )
    s8_d = nc.dram_tensor(
        "s8", [B_PER_CORE, 2, 128, 2, NIJP], FP8, kind="ExternalInput"
    ).ap()
    wvT_d = nc.dram_tensor("wvT", [128, 4, DK], BF16, kind="ExternalInput").ap()
    q_d = nc.dram_tensor(
        "qbf", [128, B_PER_CORE, 4, HW], BF16, kind="ExternalInput"
    ).ap()
    # g8 = fp8(64 * Wqk^T Wqk) in DoubleRow lhsT layout [p, g, cc', t, m]
    g8_d = nc.dram_tensor(
        "g8", [128, 2, 4, 2, 128], FP8, kind="ExternalInput"
    ).ap()
    # q8 = fp8(Q) in DoubleRow rhs layout [p, b, g, t, hw]
    q8_d = nc.dram_tensor(
        "q8", [128, B_PER_CORE, 2, 2, HW], FP8, kind="ExternalInput"
    ).ap()
    wv8_d = nc.dram_tensor("wv8", [128, 2, 2, DK], FP8, kind="ExternalInput").ap()
    out_d = nc.dram_tensor(
        "out", [128, B_PER_CORE + 1], F32, kind="ExternalOutput"
    ).ap()

    with tile.TileContext(nc) as tc:
        with (
            tc.tile_pool(name="const", bufs=1) as const,
            tc.tile_pool(name="s8p", bufs=8) as s8p,
            tc.tile_pool(name="vtp", bufs=3) as vtp,
            tc.tile_pool(name="qk8p", bufs=4) as qk8p,
            tc.tile_pool(name="small", bufs=2) as small,
            tc.tile_pool(name="ps_q", bufs=2, space="PSUM") as ps_q,
            tc.tile_pool(name="ps_vt", bufs=2, space="PSUM") as ps_vt,
            tc.tile_pool(name="ps_ctx", bufs=2, space="PSUM") as ps_ctx,
        ):
            # ---- input DMAs, ordered for fastest time-to-first-exp ----
            # Transfers FIFO-serialize at ~720B/ns in HWDGE issue order, so
            # the small q8/G8 pieces gating the qk chain go first on qSP
            # while wv8 interleaves from the qACT HWDGE queue.
            q8_sb = const.tile([128, B_PER_CORE, 2, 2, HW], FP8, tag="q8_sb")
            nc.sync.dma_start(out=q8_sb[:, 0], in_=q8_d[:, 0])
            g8_sb = const.tile([128, 2, 4, 2, 128], FP8, tag="g8_sb")
            nc.sync.dma_start(out=g8_sb, in_=g8_d)

            s8 = {}

            def s8_alloc(b):
                for g in range(2):
                    s8t = s8p.tile([128, 2, NIJP], FP8, tag="s8")
                    s8[(b, g)] = s8t

            def s8_piece(b, o, ln, eng=None):
                for g in range(2):
                    (eng or nc.sync).dma_start(
                        out=s8[(b, g)][:, :, o : o + ln],
                        in_=s8_d[b, g][:, :, o : o + ln],
                    )

            def s8_dma(b, pieces):
                s8_alloc(b)
                w = NIJP // pieces
                for i in range(pieces):
                    o = i * w
                    s8_piece(b, o, w if i < pieces - 1 else NIJP - o)

            s8_alloc(0)
            s8_alloc(1)
            wv8_sb = const.tile([128, 2, 2, DK], FP8, tag="wv8_sb")
            nc.scalar.dma_start(out=wv8_sb, in_=wv8_d)
            s8_piece(0, 0, 1280)
            s8_piece(0, 1280, 1280)
            s8_piece(0, 2560, 1280)
            s8_piece(0, 3840, 1152)
            wvT_sb = const.tile([128, 4, DK], BF16, tag="wvT_sb")
            nc.sync.dma_start(out=wvT_sb, in_=wvT_d)
            nc.sync.dma_start(out=q8_sb[:, 1:], in_=q8_d[:, 1:])
            q_sb = const.tile([128, B_PER_CORE, 4, HW], BF16, tag="q_sb")
            nc.sync.dma_start(out=q_sb[:, 0], in_=q_d[:, 0])
            s8_piece(1, 0, 1248)
            s8_piece(1, 1248, 1248)
            nc.sync.dma_start(out=q_sb[:, 1:], in_=q_d[:, 1:])
            s8_piece(1, 2496, 1248)
            s8_piece(1, 3744, 1248)

            # PE p-state warmup: wide matmuls on a zero tile start the PE
            # p-state ramp clock during the initial DMA wait so the real
            # prologue runs at or near full clock.
            e_all = const.tile(
                [128, B_PER_CORE * NCH * HW], BF16, tag="e_all"
            )
            warm_src = const.tile([128, 512], BF16, tag="warm_src")
            nc.vector.memset(warm_src, 0.0)
            # den ones vector (col 0: all rows; col 1: chunk-38 variant with
            # the 92 padded nij rows zeroed; 0-based memsets for the verifier)
            ones16 = const.tile([128, 2], BF16, tag="ones16")
            nc.vector.memset(ones16[:, 0:1], VSCALE)
            nc.vector.memset(ones16[:, 1:2], 0.0)
            nc.vector.memset(ones16[:36, 1:2], VSCALE)
            ident = const.tile([128, 128], F32, tag="ident")
            make_identity(nc, ident)
            for i in range(5):
                pw = ps_vt.tile([128, 512], F32, tag="ps_vt")
                nc.tensor.matmul(
                    pw,
                    lhsT=warm_src[:, 0:128],
                    rhs=warm_src,
                    start=True,
                    stop=True,
                )

            qv = {}

            def qv_prep(b):
                """qv[dv, hw] = Wv @ Q_b, natural orientation."""
                pt = ps_vt.tile([128, 512], F32, tag="ps_vt")
                for cc in range(4):
                    nc.tensor.matmul(
                        pt[:, 0:HW],
                        lhsT=wvT_sb[:, cc],
                        rhs=q_sb[:, b, cc],
                        start=(cc == 0),
                        stop=(cc == 3),
                    )
                qt = const.tile([128, HW], F32, tag=f"qv{b}")
                nc.vector.tensor_copy(qt, pt[:, 0:HW])
                qv[b] = qt

            qvT3 = {}

            def qvT3_prep():
                """qv^T halves for the last batch's low-latency epilogue."""
                for h in range(2):
                    hww = 128 if h == 0 else HW - 128
                    pt = ps_vt.tile([128, 512], F32, tag="ps_vt")
                    for cc in range(4):
                        nc.tensor.matmul(
                            pt[:hww, 0:128],
                            lhsT=q_sb[:, 3, cc, h * 128 : h * 128 + hww],
                            rhs=wvT_sb[:, cc],
                            start=(cc == 0),
                            stop=(cc == 3),
                        )
                    qt = const.tile([128, 128], F32, tag=f"qvT3_{h}")
                    nc.vector.tensor_copy(qt[:hww], pt[:hww, 0:128])
                    qvT3[h] = qt

            # ---- per-batch stage generators (interleavable) ----
            vt1 = {}
            qk8 = {}

            def qk_prep(b):
                """qk = 64*(Wqk^T Wqk) @ Q via host G8, straight to fp8
                DoubleRow layout [128, 2(g), 2(t), 196]; the 64x scale is
                cancelled by the exp's 1/64 scale operand. All four batches
                run in the prologue: b==0 through the ps_q pool with split
                ACT drains (fastest path to the first exp), b>=1 through
                2-wave 1-bank ps_vt tiles with DVE drains so neither the
                exp stream nor the sim-quad PSUM rotation ever waits."""
                k8 = qk8p.tile([128, 2, 2, HWP], FP8, tag="qk8")
                if b == 0:
                    pk = ps_q.tile([128, 2, 512], F32, tag="ps_q", name="pk")
                    for go in range(2):
                        for t in range(2):
                            cc = 2 * go + t
                            for gi in range(2):
                                nc.tensor.matmul(
                                    pk[:, go, t * HW : (t + 1) * HW],
                                    lhsT=g8_sb[:, gi, cc],
                                    rhs=q8_sb[:, b, gi],
                                    start=(gi == 0),
                                    stop=(gi == 1),
                                    perf_mode=DR,
                                )
                    for g in range(2):
                        nc.scalar.copy(
                            k8[:, g, :, 0:HW],
                            pk[:, g, 0 : 2 * HW].rearrange(
                                "p (t hw) -> p t hw", t=2
                            ),
                        )
                else:
                    pk = ps_q.tile([128, 2, 512], F32, tag="ps_q", name="pk")
                    for go in range(2):
                        for t in range(2):
                            cc = 2 * go + t
                            for gi in range(2):
                                nc.tensor.matmul(
                                    pk[:, go, t * HW : (t + 1) * HW],
                                    lhsT=g8_sb[:, gi, cc],
                                    rhs=q8_sb[:, b, gi],
                                    start=(gi == 0),
                                    stop=(gi == 1),
                                    perf_mode=DR,
                                )
                    nc.scalar.copy(
                        k8[:, :, :, 0:HW],
                        pk[:, :, 0 : 2 * HW].rearrange(
                            "p g (t hw) -> p g t hw", t=2
                        ),
                    )
                qk8[b] = k8

            def sim_quad_gen():
                """Yield once per sim quad: 8 DR matmuls into a 2-bank PSUM
                tile + 1 quad-fused exp on ACT. E for all 4 batches is one
                flat tile, so all 39 quads are full (they cross batch
                boundaries) and every exp is a uniform 784-column op."""
                for q in range(B_PER_CORE * NCH // 4):
                    ps = ps_q.tile([128, 2, 512], F32, tag="ps_q")
                    for ci in range(4):
                        c = 4 * q + ci
                        b, j = divmod(c, NCH)
                        for g in range(2):
                            nc.tensor.matmul(
                                ps[
                                    :,
                                    ci // 2,
                                    (ci % 2) * HW : (ci % 2) * HW + HW,
                                ],
                                lhsT=s8[(b, g)][:, :, j * 128 : (j + 1) * 128],
                                rhs=qk8[b][:, g, :, 0:HW],
                                start=(g == 0),
                                stop=(g == 1),
                                perf_mode=DR,
                            )
                    nc.scalar.activation(
                        out=e_all[:, 4 * q * HW : (4 * q + 4) * HW],
                        in_=ps[:, :, 0 : 2 * HW],
                        func=EXP,
                        scale=1.0 / 64.0,
                    )
                    yield

            def vt_alloc(b):
                vt = vtp.tile([128, NCH * 128], BF16, tag="vt1")
                vt1[b] = vt

            def vt_gen(b, j_lo, j_hi):
                """Yield per V^T chunk: 2 DoubleRow matmuls into a [128,512]
                PSUM quad tile; one wide contiguous copy per quad."""
                for j0 in range(j_lo, j_hi, 4):
                    jn = min(4, j_hi - j0)
                    pq = ps_vt.tile([128, 512], F32, tag="ps_vt")
                    for ji in range(jn):
                        j = j0 + ji
                        for g in range(2):
                            nc.tensor.matmul(
                                pq[:, ji * 128 : (ji + 1) * 128],
                                lhsT=s8[(b, g)][:, :, j * 128 : (j + 1) * 128],
                                rhs=wv8_sb[:, g],
                                start=(g == 0),
                                stop=(g == 1),
                                perf_mode=DR,
                            )
                        yield
                    nc.vector.tensor_copy(
                        vt1[b][:, j0 * 128 : (j0 + jn) * 128],
                        pq[:, 0 : jn * 128],
                    )

            def ctx_alloc(b):
                # [*, 0:196] = ctx^T (dv x hw); [*, 200:202] = den halves
                return ps_ctx.tile([128, 512], F32, tag="ps_ctx", name="pcb")

            def pv_gen(b, pcb):
                """Yield per et chunk: ctx^T matmul (lhsT = V^T chunk, rhs =
                exp chunk, both hw halves at once) + 2 one-column den
                matmuls. den and ctx carry the same 16x scale (wv8 and the
                ones vector), so recip(den)*ctx cancels it exactly."""
                for j in range(NCH):
                    nc.tensor.matmul(
                        pcb[:, 0:HW],
                        lhsT=vt1[b][:, j * 128 : (j + 1) * 128],
                        rhs=e_all[:, (b * NCH + j) * HW : (b * NCH + j + 1) * HW],
                        start=(j == 0),
                        stop=(j == NCH - 1),
                    )
                    oc = 0 if j < NCH - 1 else 1
                    nc.tensor.matmul(
                        pcb[:128, 200:201],
                        lhsT=e_all[:, (b * NCH + j) * HW : (b * NCH + j) * HW + 128],
                        rhs=ones16[:, oc : oc + 1],
                        start=(j == 0),
                        stop=(j == NCH - 1),
                    )
                    nc.tensor.matmul(
                        pcb[:68, 201:202],
                        lhsT=e_all[:, (b * NCH + j) * HW + 128 : (b * NCH + j + 1) * HW],
                        rhs=ones16[:, oc : oc + 1],
                        start=(j == 0),
                        stop=(j == NCH - 1),
                    )
                    yield
                # epilogue: r = 1/den, transposed to the free dim via the PE,
                # then partial += sum((qv - ctx*r)^2) on the DVE
                r2 = small.tile([128, 2], F32, tag="r2")
                nc.vector.reciprocal(r2[:, 0:1], pcb[:, 200:201])
                nc.vector.reciprocal(r2[:68, 1:2], pcb[:68, 201:202])
                if b == 3:
                    # last batch: transpose ctx instead of r — a shorter
                    # serial chain (ACT copy runs parallel to the recips)
                    # since this epilogue is exposed at the kernel tail
                    cx = small.tile([128, HW], F32, tag="cx")
                    nc.scalar.copy(cx, pcb[:, 0:HW])
                    pt = ps_vt.tile([128, 512], F32, tag="ps_vt", name="pt")
                    nc.tensor.matmul(
                        pt[:, 0:128],
                        lhsT=cx[:, 0:128],
                        rhs=ident,
                        is_transpose=True,
                        start=True,
                        stop=True,
                    )
                    nc.tensor.matmul(
                        pt[:68, 128:256],
                        lhsT=cx[:, 128:HW],
                        rhs=ident,
                        is_transpose=True,
                        start=True,
                        stop=True,
                    )
                    for h in range(2):
                        hww = 128 if h == 0 else HW - 128
                        dh = small.tile([128, 128], F32, tag=f"dh{h}")
                        nc.vector.scalar_tensor_tensor(
                            dh[:hww],
                            pt[:hww, h * 128 : h * 128 + 128],
                            r2[:hww, h : h + 1],
                            qvT3[h][:hww],
                            op0=MULT,
                            op1=SUBTRACT,
                        )
                        d2h = small.tile([128, 128], F32, tag=f"d2h{h}")
                        nc.vector.scalar_tensor_tensor(
                            d2h[:hww],
                            dh[:hww],
                            1.0,
                            dh[:hww],
                            op0=MULT,
                            op1=MULT,
                            accum_out=partials[:hww, 3 + h : 4 + h],
                        )
                    return
                rr = ps_vt.tile([128, 512], F32, tag="ps_vt", name="rr")
                nc.tensor.matmul(
                    rr[0:1, 0:128],
                    lhsT=r2[:, 0:1],
                    rhs=ident[:, 0:128],
                    is_transpose=True,
                    start=True,
                    stop=True,
                )
                nc.tensor.matmul(
                    rr[0:1, 128:HW],
                    lhsT=r2[:68, 1:2],
                    rhs=ident[0:68, 0:68],
                    is_transpose=True,
                    start=True,
                    stop=True,
                )
                rs = small.tile([128, HW], F32, tag="rs")
                nc.vector.tensor_copy(rs[0:1, :], rr[0:1, 0:HW])
                rfull = small.tile([128, HW], F32, tag="rfull")
                nc.gpsimd.partition_broadcast(rfull, rs[0:1, :])
                t1 = small.tile([128, HW], F32, tag="t1")
                nc.vector.scalar_tensor_tensor(
                    t1,
                    pcb[:, 0:HW],
                    1.0,
                    rfull,
                    op0=MULT,
                    op1=MULT,
                )
                d = small.tile([128, HW], F32, tag="d")
                nc.vector.tensor_sub(d, t1, qv[b])
                d2 = small.tile([128, HW], F32, tag="d2")
                nc.vector.scalar_tensor_tensor(
                    d2,
                    d,
                    1.0,
                    d,
                    op0=MULT,
                    op1=MULT,
                    accum_out=partials[:, b : b + 1],
                )

            def drain(gen, n=None):
                if gen is None:
                    return None
                try:
                    if n is None:
                        while True:
                            next(gen)
                    else:
                        for _ in range(n):
                            next(gen)
                except StopIteration:
                    return None
                return gen

            # ---- schedule ----
            # PV(b) runs in-iteration, lagging its own exp stream by 2 slots;
            # its tail + ctx epilogue carry into iteration b+1.
            partials = const.tile([128, B_PER_CORE + 1], F32, tag="partials")
            nc.vector.memset(partials, 0.0)
            qk_prep(0)
            vt_alloc(0)
            drain(vt_gen(0, 0, 8))      # keeps the PE busy until qk8 lands
            vt0_rest = vt_gen(0, 8, NCH)

            simg = sim_quad_gen()
            carry = None        # pv tail from the previous batch
            vt_carry = None     # vt tail from the previous batch
            pvg = None

            for s in range(B_PER_CORE * NQ):
                b, jq = divmod(s, NQ)
                simg = drain(simg, 1)
                if jq == 0:
                    pcb = ctx_alloc(b)
                    pvg = pv_gen(b, pcb)
                    if b + 1 < B_PER_CORE:
                        vt_alloc(b + 1)
                        vtg = vt_gen(b + 1, 0, NCH)
                    else:
                        vtg = None
                    if b + 2 < B_PER_CORE:
                        s8_dma(b + 2, 4)
                if b == 0:
                    vt0_rest = drain(vt0_rest, 4)
                if jq == 4 and b + 1 < B_PER_CORE:
                    qk_prep(b + 1)
                if jq < 2:
                    if vt_carry is not None:
                        vt_carry = drain(vt_carry, 3)
                    if carry is not None:
                        carry = drain(carry, 4)
                if b == 0:
                    if jq >= 5:
                        vtg = drain(vtg, 7)
                elif jq >= 1:
                    vtg = drain(vtg, 4)
                if jq >= 3:
                    pvg = drain(pvg, 5)
                if jq == 7 and b < 3:
                    qv_prep(b)
                if jq == 5 and b == 3:
                    qvT3_prep()
                if jq == NQ - 1:
                    vt_carry = vtg
                    carry = pvg

            drain(carry)

            # final reduction happens on the host: DMA the partials matrix
            nc.sync.dma_start(out=out_d, in_=partials)

    nc.compile()
    return nc


_NC = None


def _prep_core(q, s, b0):
    """Host-side per-core input layouts (cast + transpose + zero-pad only)."""
    F8NP = ml_dtypes.float8_e4m3
    BFNP = ml_dtypes.bfloat16
    sb = s[b0 : b0 + B_PER_CORE]                      # [4, 25, 512, 196]
    s_c = sb.transpose(0, 2, 1, 3).reshape(B_PER_CORE, 2, 2, 128, NIJ)
    s8 = np.zeros((B_PER_CORE, 2, 128, 2, NIJP), dtype=F8NP)
    s8[:, :, :, :, :NIJ] = s_c.transpose(0, 1, 3, 2, 4).astype(F8NP)
    qb = q[b0 : b0 + B_PER_CORE]                      # [4, 512, 196]
    qbf = np.ascontiguousarray(
        qb.reshape(B_PER_CORE, 4, 128, HW).transpose(2, 0, 1, 3)
    ).astype(BFNP)                                    # [128, b, cc, hw]
    q8 = np.ascontiguousarray(
        qb.reshape(B_PER_CORE, 2, 2, 128, HW).transpose(3, 0, 1, 2, 4)
    ).astype(F8NP)                                    # [128, b, g, t, hw]
    return {"s8": s8, "qbf": qbf, "q8": q8}


def kernel(query_repr, supports_repr, W_qk, W_v):
    global _NC
    F8NP = ml_dtypes.float8_e4m3
    BFNP = ml_dtypes.bfloat16

    q = np.asarray(query_repr, dtype=np.float32).reshape(32, C, HW)
    s = np.asarray(supports_repr, dtype=np.float32).reshape(32, N_SUP, C, HW)
    wqk = np.asarray(W_qk, dtype=np.float32)
    wv = np.asarray(W_v, dtype=np.float32)

    wvT = np.ascontiguousarray(
        wv.T.reshape(4, 128, DK).transpose(1, 0, 2)
    ).astype(BFNP)
    wv8 = np.ascontiguousarray(
        (VSCALE * wv).T.reshape(2, 2, 128, DK).transpose(2, 0, 1, 3)
    ).astype(F8NP)
    # G8 = 64 * Wqk^T Wqk (symmetric), fp8 DoubleRow lhsT layout
    g = 64.0 * (wqk.T @ wqk)                          # [512, 512]
    g8 = np.ascontiguousarray(
        g.reshape(2, 2, 128, 4, 128).transpose(2, 0, 3, 1, 4)
    ).astype(F8NP)                                    # [p, g, cc', t, m]

    if _NC is None:
        _NC = build_bass()

    in_maps = []
    for core in range(8):
        m = _prep_core(q, s, core * B_PER_CORE)
        m.update({"wvT": wvT, "wv8": wv8, "g8": g8})
        in_maps.append(m)
    res = run_bass_kernel_spmd(
        _NC, in_maps, core_ids=list(range(8)),
        trace=bool(int(os.environ.get("KTRACE", "0"))),
    )
    total = sum(float(r["out"].astype(np.float64).sum()) for r in res.results)
    total = total / float(HW)
    kernel._last_results = res
    return np.asarray(total, dtype=np.float32)
